# revision 29
# baseline (speedup 1.0000x reference)
"""Causal self-attention (B=4, T=2048, C=1024, H=16, Dh=64) on 8 trn2 NeuronCores.

Sharding: core i <-> (batch b = i//2, head-group g = i%2). Each core computes
8 heads of one batch end-to-end (qkv slice, causal attention, partial output
projection); the host sums the head-group/pair-couple partials per batch and
adds bproj. No device collectives.

x arrives host-pretransposed as xT[C, T] (fp16), so qkv matmuls stream it
directly with the contraction dim on partitions -- no on-device transposes.
Attention uses the transposed-scores layout sT[tk, tq]: softmax denominators
come out of the PV matmul via an extra ones column interleaved into Wv, and
are broadcast across partitions with a partition-step-0 SBUF->SBUF DMA.
Partial projection outputs are written fp16 and summed on the host.
"""

import numpy as np

import concourse.bass as bass
import concourse.tile as tile
from concourse import bacc, mybir
from concourse.bass_utils import run_bass_kernel_spmd

F32 = mybir.dt.float32
F32R = mybir.dt.float32r
F16 = mybir.dt.float16

N_CORES = 8
B, T, C = 4, 2048, 1024
NH_TOT, D = 16, 64
F = 512            # features per core (8 heads)
NH = 8             # local heads
NPAIR = 4          # head pairs (128 feats each)
CCH = C // 128     # 8 contraction chunks
NTT = T // 128     # 16 t tiles
NTB = T // 512     # 4 t blocks (qkv production)
NQB = T // 512     # 4 q blocks (attention)
VW = NH * (D + 1)  # 520: augmented v width
ADD = mybir.AluOpType.add
MULT = mybir.AluOpType.mult


def _emit(tc, aps):
    from contextlib import ExitStack
    nc = tc.nc
    x, wq, wk, wva, bq, bk, wp = (
        aps["x"], aps["wq"], aps["wk"], aps["wva"], aps["bq"], aps["bk"],
        aps["wp"])
    cmask = aps["cmask"]
    out_ab = [aps["out_pa"], aps["out_pb"]]

    # ---- pools (all coexist; ~210KB/partition total) ----
    ctx = ExitStack()
    pp_qk = ctx.enter_context(tc.tile_pool(name="ps_qk", bufs=2, space="PSUM"))
    pp_s = ctx.enter_context(tc.tile_pool(name="ps_s", bufs=2, space="PSUM"))
    pp_pv = ctx.enter_context(tc.tile_pool(name="ps_pv", bufs=1, space="PSUM"))
    po_v = ctx.enter_context(tc.tile_pool(name="v_all", bufs=1))
    po_mask = ctx.enter_context(tc.tile_pool(name="mask", bufs=1))
    po_wv = ctx.enter_context(tc.tile_pool(name="wv", bufs=16))
    po_qkt = ctx.enter_context(tc.tile_pool(name="qkT", bufs=2))
    po_bias = ctx.enter_context(tc.tile_pool(name="bias", bufs=1))
    po_misc = ctx.enter_context(tc.tile_pool(name="misc", bufs=3))
    po_xt = ctx.enter_context(tc.tile_pool(name="xT", bufs=1))
    po_wqk = ctx.enter_context(tc.tile_pool(name="wqk", bufs=8))
    po_yt = ctx.enter_context(tc.tile_pool(name="yT", bufs=4))
    po_exp = ctx.enter_context(tc.tile_pool(name="expT", bufs=4))
    po_rec = ctx.enter_context(tc.tile_pool(name="recip", bufs=3))
    po_den = ctx.enter_context(tc.tile_pool(name="den", bufs=2))
    po_ytmp = ctx.enter_context(tc.tile_pool(name="ytmp", bufs=2))
    po_wp = ctx.enter_context(tc.tile_pool(name="wp", bufs=4))
    po_dram = ctx.enter_context(tc.tile_pool(name="dram_scr", bufs=4,
                                             space="DRAM"))

    mask_sb = po_mask.tile([128, 512], F32, tag="mask")
    nc.sync.dma_start(out=mask_sb[:], in_=cmask[:])
    # bva broadcast to all 128 partitions straight from DRAM
    bva_bc = po_bias.tile([128, VW], F32, tag="bva_bc")
    bva2 = aps["bva2"]
    nc.sync.dma_start(out=bva_bc[:], in_=bass.AP(
        tensor=bva2.tensor, offset=bva2.offset,
        ap=[[0, 128]] + [list(a) for a in bva2.ap[1:]]))

    # ---- phase 0: load host-pretransposed xT [C, T] + wv per chunk, so
    # the first v matmul only waits on chunk 0 (wv on the scalar HWDGE
    # ring, xT on the sync ring -- two rings run in parallel)
    xT = [po_xt.tile([128, T], F16, tag=f"xT{c}", name=f"xT{c}")
          for c in range(CCH)]
    wv_sb = [[None] * CCH, [None] * CCH]
    for c in range(CCH):
        nc.sync.dma_start(out=xT[c][:], in_=x[c * 128:(c + 1) * 128, :])
        for half in range(2):
            cs = slice(half * 260, half * 260 + 260)
            wt = po_wv.tile([128, 260], F16, tag="wv")
            nc.scalar.dma_start(out=wt[:], in_=wva[c * 128:(c + 1) * 128, cs])
            wv_sb[half][c] = wt

    # ---- phase 0b: v (augmented with ones columns, all 8 heads) ----
    # half 0 = heads 0-3 (pairs 0,1), half 1 = heads 4-7 (pairs 2,3);
    # half 1 production overlaps pair-0 attention. tt pairs alternate
    # psum banks so consecutive matmuls never accumulate into the same
    # bank back-to-back
    v_all = [po_v.tile([128, VW], F16, tag=f"v{tt}", name=f"v{tt}")
             for tt in range(NTT)]

    def v_units(half, split=1):
        cs = slice(half * 260, half * 260 + 260)
        units = []
        for tt0 in range(0, NTT, 2):
            stt = {}

            def part(tt0=tt0, cs=cs, half=half, stt=stt, c0=0, c1=CCH,
                     fin=True):
                if c0 == 0:
                    stt["ps0"] = pp_qk.tile([128, 260], F32, tag="qk",
                                            name="ps0")
                    stt["ps1"] = pp_qk.tile([128, 260], F32, tag="qk",
                                            name="ps1")
                ps0, ps1 = stt["ps0"], stt["ps1"]
                for c in range(c0, c1):
                    nc.tensor.matmul(
                        ps0[:], xT[c][:, tt0 * 128:(tt0 + 1) * 128],
                        wv_sb[half][c][:], start=(c == 0),
                        stop=(c == CCH - 1))
                    nc.tensor.matmul(
                        ps1[:], xT[c][:, (tt0 + 1) * 128:(tt0 + 2) * 128],
                        wv_sb[half][c][:], start=(c == 0),
                        stop=(c == CCH - 1))
                if fin:
                    nc.vector.tensor_add(v_all[tt0][:, cs], ps0[:],
                                         bva_bc[:, cs])
                    nc.vector.tensor_add(v_all[tt0 + 1][:, cs], ps1[:],
                                         bva_bc[:, cs])

            if split == 1:
                units.append(part)
            else:
                from functools import partial
                units.append(partial(part, c0=0, c1=4, fin=False))
                units.append(partial(part, c0=4, c1=CCH, fin=True))
        return units

    # ---- per head pair: qkv -> attention -> partial proj ----
    # Emitted as interleaved work units so the PE instruction stream mixes
    # next-pair qkv (and couple proj) matmuls between attention groups --
    # engines are in-order, so a blocked exp-wait would otherwise stall
    # ready qkv work behind it.

    def prep_qkv(pair):
        psl = slice(pair * 128, (pair + 1) * 128)
        wqk_c = []
        for c in range(CCH):
            wt = po_wqk.tile([128, 256], F16, tag="wqk", name="wt")
            nc.sync.dma_start(out=wt[:, 0:128],
                              in_=wq[c * 128:(c + 1) * 128, psl])
            nc.sync.dma_start(out=wt[:, 128:256],
                              in_=wk[c * 128:(c + 1) * 128, psl])
            wqk_c.append(wt)
        bq_sb = po_bias.tile([128, 1], F32, tag=f"bq{pair}", name=f"bq{pair}")
        nc.sync.dma_start(out=bq_sb[:], in_=bq[psl, :])
        bk_sb = po_bias.tile([128, 1], F32, tag=f"bk{pair}", name=f"bk{pair}")
        nc.sync.dma_start(out=bk_sb[:], in_=bk[psl, :])
        qT = po_qkt.tile([128, T], F16, tag="qT", name="qT")
        kT = po_qkt.tile([128, T], F16, tag="kT", name="kT")
        return dict(wqk=wqk_c, bq=bq_sb, bk=bk_sb, qT=qT, kT=kT)

    def qkv_units(st8, split=1):
        # split=2 yields two micro-closures per t-block (for weaving
        # between attention steps); psum tile lifetime spans the pair,
        # so micros of one t-block must stay adjacent in their stream
        units = []
        for tb in range(NTB):
            tsl = slice(tb * 512, (tb + 1) * 512)
            stt = {}

            def half(tb=tb, tsl=tsl, stt=stt, c0=0, c1=CCH, fin=True):
                if c0 == 0:
                    stt["psq"] = pp_qk.tile([128, 512], F32, tag="qk",
                                            name="psq")
                    stt["psk"] = pp_qk.tile([128, 512], F32, tag="qk",
                                            name="psk")
                psq, psk = stt["psq"], stt["psk"]
                # q/k matmuls interleaved so consecutive matmuls target
                # alternating psum banks
                for c in range(c0, c1):
                    nc.tensor.matmul(psq[:], st8["wqk"][c][:, 0:128],
                                     xT[c][:, tsl],
                                     start=(c == 0), stop=(c == CCH - 1))
                    nc.tensor.matmul(psk[:], st8["wqk"][c][:, 128:256],
                                     xT[c][:, tsl],
                                     start=(c == 0), stop=(c == CCH - 1))
                if fin:
                    # psum*1/sqrt(D) + bq/sqrt(D)  (bq pre-scaled on host)
                    nc.vector.tensor_scalar(
                        out=st8["qT"][:, tsl], in0=psq[:], scalar1=0.125,
                        scalar2=st8["bq"][:], op0=MULT, op1=ADD)
                    nc.vector.tensor_scalar(
                        out=st8["kT"][:, tsl], in0=psk[:],
                        scalar1=st8["bk"][:], scalar2=None, op0=ADD)

            if split == 1:
                units.append(half)
            else:
                from functools import partial
                units.append(partial(half, c0=0, c1=4, fin=False))
                units.append(partial(half, c0=4, c1=CCH, fin=True))
        return units

    def attn_micros(st8, yt):
        # One q-block unit computes BOTH heads of the pair: the two
        # score matmuls contract over disjoint 64-partition halves
        # (rows 0-63 = head hl0, rows 64-127 = head hl1) so the PE runs
        # them CONCURRENTLY in separate row groups. One st tile packs
        # [hl0 scores | hl1 scores] so a single wide exp covers both.
        # Returns kt-granular micro-closures (for weaving fillers into
        # the exp-latency slack) plus per-unit end indices for gating.
        qT, kT = st8["qT"], st8["kT"]
        micros = []
        ends = []
        for qb in range(NQB):
            nkt = 4 * qb + 4
            ustate = {}

            def emit_scores(kt, st, qb=qb):
                j = kt - 4 * qb
                off = 128 * j if j > 0 else 0
                ktw = slice(kt * 128, (kt + 1) * 128)
                qw = slice(qb * 512 + off, (qb + 1) * 512)
                nc.tensor.matmul(st[:, off:512], kT[0:64, ktw],
                                 qT[0:64, qw], start=True, stop=True)
                nc.tensor.matmul(st[:, 512 + off:1024], kT[64:128, ktw],
                                 qT[64:128, qw], start=True, stop=True)
                if j >= 0:
                    # only the first 128 cols of the valid window can be
                    # sub-causal (u' < p <= 128)
                    nc.vector.tensor_add(
                        st[:, off:off + 128], st[:, off:off + 128],
                        mask_sb[:, 0:128])
                    nc.vector.tensor_add(
                        st[:, 512 + off:512 + off + 128],
                        st[:, 512 + off:512 + off + 128],
                        mask_sb[:, 0:128])
                return off

            def emit_exp_pv(kt, st, off, qb=qb, nkt=nkt, ustate=ustate):
                pair = st8["pair"]
                vslA = slice((pair * 2) * 65, (pair * 2) * 65 + 65)
                vslB = slice((pair * 2 + 1) * 65, (pair * 2 + 1) * 65 + 65)
                et = po_exp.tile([128, 1024], F16, tag="expT", name="et")
                if off > 0:
                    # never-matmul'd strip between the two head windows
                    nc.vector.memset(st[:, 512:512 + off], -1e9)
                nc.scalar.activation(
                    et[:, off:1024], st[:, off:1024],
                    mybir.ActivationFunctionType.Exp)
                nc.tensor.matmul(ustate["pvA"][0:65, off:512],
                                 v_all[kt][:, vslA], et[:, off:512],
                                 start=(kt == 0), stop=(kt == nkt - 1))
                nc.tensor.matmul(ustate["pvB"][0:65, off:512],
                                 v_all[kt][:, vslB], et[:, 512 + off:1024],
                                 start=(kt == 0), stop=(kt == nkt - 1))

            def step(kt, qb=qb, nkt=nkt, ustate=ustate, es=emit_scores,
                     ep=emit_exp_pv):
                if kt == 0:
                    ustate["pvA"] = pp_pv.tile([128, 512], F32, tag="pvA",
                                               name="pvA")
                    ustate["pvB"] = pp_pv.tile([128, 512], F32, tag="pvB",
                                               name="pvB")
                if kt < nkt:
                    st = pp_s.tile([128, 1024], F32, tag="s", name="st")
                    off = es(kt, st, qb=qb)
                    pend = ustate.get("pend")
                    if pend is not None:
                        ep(*pend)
                    ustate["pend"] = (kt, st, off)
                else:
                    ep(*ustate["pend"])
                    ustate["pend"] = None

            def tail(hl, qb=qb, ustate=ustate):
                # den row 64 -> DRAM bounce broadcast -> recip -> mul
                qsl = slice(qb * 512, (qb + 1) * 512)
                pv = ustate["pvA"] if hl == 0 else ustate["pvB"]
                den = po_den.tile([128, 512], F32, tag="den", name="den")
                nc.vector.tensor_copy(den[64:65, :], pv[64:65, :])
                dscr = po_dram.tile([1, 512], F32, tag="dscr", name="dscr")
                # gpsimd SWDGE queue: independent of the bulk-load/store
                # HWDGE rings, so these latency-critical hops never queue
                # behind 256KB transfers
                nc.gpsimd.dma_start(out=dscr[:], in_=den[64:65, :])
                rec = po_rec.tile([128, 512], F32, tag="recip", name="rec")
                nc.gpsimd.dma_start(out=rec[0:64, :], in_=bass.AP(
                    tensor=dscr.tensor, offset=dscr[:].offset,
                    ap=[[0, 64]] + [list(a) for a in dscr[:].ap[1:]]))
                nc.vector.reciprocal_approx_fast(rec[0:64, :], rec[0:64, :])
                if hl == 0:
                    nc.vector.tensor_mul(yt[0:64, qsl], pv[0:64, :],
                                         rec[0:64, :])
                else:
                    # engines can't cross partitions; bounce via DMA
                    ytmp = po_ytmp.tile([128, 512], F16, tag="ytmp",
                                        name="ytmp")
                    nc.vector.tensor_mul(ytmp[0:64, :], pv[0:64, :],
                                         rec[0:64, :])
                    nc.gpsimd.dma_start(out=yt[64:128, qsl],
                                        in_=ytmp[0:64, :])

            from functools import partial
            for kt in range(nkt):
                micros.append(partial(step, kt))
            def last(ustate=ustate, step=step, tail=tail, nkt=nkt):
                step(nkt)
                tail(0)
            micros.append(last)
            micros.append(partial(tail, 1))
            ends.append(len(micros) - 1)
        return micros, ends

    def prep_proj(couple):
        wp_sb = []
        for pq in range(2):
            for cb in range(2):
                prow = (couple * 2 + pq) * 128
                wt = po_wp.tile([128, 512], F16, tag="wp", name="wpt")
                nc.sync.dma_start(
                    out=wt[:],
                    in_=wp[prow:prow + 128, cb * 512:(cb + 1) * 512])
                wp_sb.append(wt)
        return wp_sb

    def proj_micros(couple, wp_sb, yts, tts):
        out_p = out_ab[couple]
        micros = []
        for tt in tts:
            def micro(tt=tt):
                ot = po_misc.tile([128, C], F16, tag="misc", name="ot")
                ps0 = pp_qk.tile([128, 512], F32, tag="qk", name="pp0")
                ps1 = pp_qk.tile([128, 512], F32, tag="qk", name="pp1")
                pss = (ps0, ps1)
                # pq outer: the y stationary is reused across the two
                # cb matmuls, which alternate psum banks
                for pq in range(2):
                    for cb in range(2):
                        nc.tensor.matmul(
                            pss[cb][:],
                            yts[pq][:, tt * 128:(tt + 1) * 128],
                            wp_sb[pq * 2 + cb][:],
                            start=(pq == 0), stop=(pq == 1))
                for cb in range(2):
                    nc.vector.tensor_copy(
                        ot[:, cb * 512:(cb + 1) * 512], pss[cb][:])
                nc.sync.dma_start(
                    out=out_p[tt * 128:(tt + 1) * 128, :], in_=ot[:])
            micros.append(micro)
        return micros

    def round_robin(*streams):
        streams = [list(s) for s in streams if s]
        while any(streams):
            for s in streams:
                if s:
                    s.pop(0)()

    def weave(primary, fillers, gates=None):
        # Spread filler micro-closures evenly between primary ones;
        # gates[i] = index into primary that must already be emitted
        # before fillers[i] may run.
        nf, npr = len(fillers), len(primary)
        r = nf / npr if npr else 0.0
        acc, fi = 0.0, 0
        for pi, u in enumerate(primary):
            u()
            acc += r
            while (fi < nf and acc >= 1.0
                   and (gates is None or gates[fi] <= pi)):
                fillers[fi]()
                fi += 1
                acc -= 1.0
        while fi < nf:
            fillers[fi]()
            fi += 1

    pair_state = []
    yts = []
    st0 = prep_qkv(0)
    st0["pair"] = 0
    pair_state.append(st0)
    # pair-0 qkv races ahead of v-half0 so pair-0 attention (which only
    # needs half-0 v) starts as early as possible; v-half1 (for pairs
    # 2,3) overlaps pair-0 attention
    round_robin(qkv_units(st0), v_units(0))
    wp_sb0 = wp_sb1 = None
    for p in range(NPAIR):
        yt = po_yt.tile([128, T], F16, tag="yT", name="yt")
        yts.append(yt)
        am, ends = attn_micros(pair_state[p], yt)
        fillers = []
        gates = []
        if p + 1 < NPAIR:
            stn = prep_qkv(p + 1)
            stn["pair"] = p + 1
            pair_state.append(stn)
            qm = qkv_units(stn, split=2)
            fillers += qm
            gates += [-1] * len(qm)
        if p == 0:
            vm = v_units(1, split=2)
            fillers += vm
            gates += [-1] * len(vm)
        if p == 1:
            # couple-0 proj: first half here, gated on the attn-1
            # q-blocks that complete the yts[1] columns it reads
            wp_sb0 = prep_proj(0)
            pm = proj_micros(0, wp_sb0, yts[0:2], range(0, 8))
            fillers += pm
            gates += [ends[tt // 4] for tt in range(0, 8)]
        if p == 2:
            pm = proj_micros(0, wp_sb0, yts[0:2], range(8, 16))
            fillers += pm
            gates += [-1] * len(pm)
        if p == 3:
            wp_sb1 = prep_proj(1)
            pm = proj_micros(1, wp_sb1, yts[2:4], range(0, 16))
            fillers += pm
            gates += [ends[tt // 4] for tt in range(0, 16)]
        weave(am, fillers, gates)

    ctx.close()


_CACHE = {}


def _build():
    if "nc" in _CACHE:
        return _CACHE["nc"]
    nc = bacc.Bacc("TRN2", target_bir_lowering=False, debug=False,
                   enable_asserts=True, num_devices=N_CORES)
    aps = {
        "x": nc.dram_tensor("x", [C, T], F16, kind="ExternalInput").ap(),
        "wq": nc.dram_tensor("wq", [C, F], F16, kind="ExternalInput").ap(),
        "wk": nc.dram_tensor("wk", [C, F], F16, kind="ExternalInput").ap(),
        "wva": nc.dram_tensor("wva", [C, VW], F16, kind="ExternalInput").ap(),
        "bq": nc.dram_tensor("bq", [F, 1], F32, kind="ExternalInput").ap(),
        "bk": nc.dram_tensor("bk", [F, 1], F32, kind="ExternalInput").ap(),
        "bva2": nc.dram_tensor("bva2", [1, VW], F32, kind="ExternalInput").ap(),
        "wp": nc.dram_tensor("wp", [F, C], F16, kind="ExternalInput").ap(),
        "cmask": nc.dram_tensor("cmask", [128, 512], F32,
                                kind="ExternalInput").ap(),
        "out_pa": nc.dram_tensor("out_pa", [T, C], F16,
                                 kind="ExternalOutput").ap(),
        "out_pb": nc.dram_tensor("out_pb", [T, C], F16,
                                 kind="ExternalOutput").ap(),
    }
    with tile.TileContext(nc) as tc:
        _emit(tc, aps)
    nc.compile()
    _CACHE["nc"] = nc
    return nc


def _make_in_maps(x, Wqkv, bqkv, Wproj):
    x = np.asarray(x, dtype=np.float32)
    Wqkv = np.asarray(Wqkv, dtype=np.float32)
    bqkv = np.asarray(bqkv, dtype=np.float32)
    Wproj = np.asarray(Wproj, dtype=np.float32)

    # triangular causal mask: M[p, f] = 0 if f >= p else -1e9
    p_idx = np.arange(128)[:, None]
    u_idx = np.arange(512)[None, :]
    cmask = np.where(u_idx >= p_idx, 0.0, -1e9).astype(np.float32)

    in_maps = []
    for core in range(N_CORES):
        b, g = divmod(core, 2)
        q0, k0, v0 = 512 * g, C + 512 * g, 2 * C + 512 * g
        wva = np.zeros((C, VW), dtype=np.float32)
        bva = np.zeros((1, VW), dtype=np.float32)
        for h in range(NH):
            src = v0 + D * h
            dst = 65 * h
            # per-head layout [v(64), one]
            wva[:, dst:dst + 64] = Wqkv[:, src:src + 64]
            bva[0, dst:dst + 64] = bqkv[src:src + 64]
            bva[0, dst + 64] = 1.0
        in_maps.append({
            "x": np.ascontiguousarray(x[b].T).astype(np.float16),
            "wq": np.ascontiguousarray(Wqkv[:, q0:q0 + F]).astype(np.float16),
            "wk": np.ascontiguousarray(Wqkv[:, k0:k0 + F]).astype(np.float16),
            "wva": wva.astype(np.float16),
            "bq": np.ascontiguousarray(bqkv[q0:q0 + F].reshape(F, 1) * 0.125),
            "bk": np.ascontiguousarray(bqkv[k0:k0 + F].reshape(F, 1)),
            "bva2": bva,
            "wp": np.ascontiguousarray(Wproj[512 * g:512 * g + F, :]).astype(np.float16),
            "cmask": cmask,
        })
    return in_maps


def run_sharded(x, Wqkv, bqkv, Wproj, bproj, trace=False):
    nc = _build()
    in_maps = _make_in_maps(x, Wqkv, bqkv, Wproj)
    res = run_bass_kernel_spmd(nc, in_maps, core_ids=list(range(N_CORES)),
                               trace=trace)
    bproj = np.asarray(bproj, dtype=np.float32)
    out = np.empty((B, T, C), dtype=np.float32)
    for b in range(B):
        acc = bproj[None, :].astype(np.float32).repeat(T, axis=0)
        for core in (2 * b, 2 * b + 1):
            acc = (acc + res.results[core]["out_pa"].astype(np.float32)
                   + res.results[core]["out_pb"].astype(np.float32))
        out[b] = acc
    return out, res


def kernel(x, Wqkv, bqkv, Wproj, bproj):
    out, _ = run_sharded(x, Wqkv, bqkv, Wproj, bproj, trace=False)
    return out



# revision 37
# speedup vs baseline: 1.0092x; 1.0092x over previous
"""Causal self-attention (B=4, T=2048, C=1024, H=16, Dh=64) on 8 trn2 NeuronCores.

Sharding: core i <-> (batch b = i//2, head-group g = i%2). Each core computes
8 heads of one batch end-to-end (qkv slice, causal attention, partial output
projection); the host sums the head-group/pair-couple partials per batch and
adds bproj. No device collectives.

x arrives host-pretransposed as xT[C, T] (fp16), so qkv matmuls stream it
directly with the contraction dim on partitions -- no on-device transposes.
Attention uses the transposed-scores layout sT[tk, tq]: softmax denominators
come out of the PV matmul via an extra ones column interleaved into Wv, and
are broadcast across partitions with a partition-step-0 SBUF->SBUF DMA.
Partial projection outputs are written fp16 and summed on the host.
"""

import numpy as np

import concourse.bass as bass
import concourse.tile as tile
from concourse import bacc, mybir
from concourse.bass_utils import run_bass_kernel_spmd

F32 = mybir.dt.float32
F32R = mybir.dt.float32r
F16 = mybir.dt.float16

N_CORES = 8
B, T, C = 4, 2048, 1024
NH_TOT, D = 16, 64
F = 512            # features per core (8 heads)
NH = 8             # local heads
NPAIR = 4          # head pairs (128 feats each)
CCH = C // 128     # 8 contraction chunks
NTT = T // 128     # 16 t tiles
NTB = T // 512     # 4 t blocks (qkv production)
NQB = T // 512     # 4 q blocks (attention)
VW = NH * (D + 1)  # 520: augmented v width
ADD = mybir.AluOpType.add
MULT = mybir.AluOpType.mult


def _emit(tc, aps):
    from contextlib import ExitStack
    nc = tc.nc
    x, wq, wk, wva, bq, bk, wp = (
        aps["x"], aps["wq"], aps["wk"], aps["wva"], aps["bq"], aps["bk"],
        aps["wp"])
    out_ab = [aps["out_pa"], aps["out_pb"]]

    # ---- pools (all coexist; ~210KB/partition total) ----
    ctx = ExitStack()
    pp_qk = ctx.enter_context(tc.tile_pool(name="ps_qk", bufs=2, space="PSUM"))
    pp_s = ctx.enter_context(tc.tile_pool(name="ps_s", bufs=2, space="PSUM"))
    pp_pv = ctx.enter_context(tc.tile_pool(name="ps_pv", bufs=1, space="PSUM"))
    po_v = ctx.enter_context(tc.tile_pool(name="v_all", bufs=1))
    po_mask = ctx.enter_context(tc.tile_pool(name="mask", bufs=1))
    po_wv = ctx.enter_context(tc.tile_pool(name="wv", bufs=16))
    po_qkt = ctx.enter_context(tc.tile_pool(name="qkT", bufs=2))
    po_bias = ctx.enter_context(tc.tile_pool(name="bias", bufs=1))
    po_misc = ctx.enter_context(tc.tile_pool(name="misc", bufs=3))
    po_xt = ctx.enter_context(tc.tile_pool(name="xT", bufs=1))
    po_wqk = ctx.enter_context(tc.tile_pool(name="wqk", bufs=8))
    po_yt = ctx.enter_context(tc.tile_pool(name="yT", bufs=4))
    po_exp = ctx.enter_context(tc.tile_pool(name="expT", bufs=4))
    po_rec = ctx.enter_context(tc.tile_pool(name="recip", bufs=3))
    po_den = ctx.enter_context(tc.tile_pool(name="den", bufs=2))
    po_ytmp = ctx.enter_context(tc.tile_pool(name="ytmp", bufs=2))
    po_wp = ctx.enter_context(tc.tile_pool(name="wp", bufs=4))
    po_dram = ctx.enter_context(tc.tile_pool(name="dram_scr", bufs=4,
                                             space="DRAM"))

    # emask[p, w] = -6e4 if w < 384 + p else 0: sliced at [384-off, 512)
    # it masks the diagonal 128-col triangle plus the garbage strip, and
    # is ACCUMULATED onto scores by the PE itself (identity stationary)
    # so the softmax chain never hops through the vector engine
    emask_sb = po_mask.tile([128, 512], F16, tag="emask")
    nc.scalar.dma_start(out=emask_sb[:], in_=aps["emask"][:])
    ident = po_bias.tile([128, 128], F16, tag="ident")
    nc.scalar.dma_start(out=ident[:], in_=aps["identin"][:])
    # bva broadcast to all 128 partitions straight from DRAM
    bva_bc = po_bias.tile([128, VW], F32, tag="bva_bc")
    bva2 = aps["bva2"]
    nc.scalar.dma_start(out=bva_bc[:], in_=bass.AP(
        tensor=bva2.tensor, offset=bva2.offset,
        ap=[[0, 128]] + [list(a) for a in bva2.ap[1:]]))

    # ---- phase 0: small weights first (pair-0 qkv + wv), then the 4MB
    # xT so the first qkv/v matmuls unblock as early as possible; wv on
    # the scalar HWDGE ring, xT on the sync ring -- rings run in parallel
    xT = [po_xt.tile([128, T], F16, tag=f"xT{c}", name=f"xT{c}")
          for c in range(CCH)]
    wv_sb = [[None] * CCH, [None] * CCH]
    for c in range(CCH):
        for half in range(2):
            cs = slice(half * 260, half * 260 + 260)
            wt = po_wv.tile([128, 260], F16, tag="wv")
            nc.scalar.dma_start(out=wt[:], in_=wva[c * 128:(c + 1) * 128, cs])
            wv_sb[half][c] = wt

    def load_xt():
        for c in range(CCH):
            nc.sync.dma_start(out=xT[c][:], in_=x[c * 128:(c + 1) * 128, :])

    # ---- phase 0b: v (augmented with ones columns, all 8 heads) ----
    # half 0 = heads 0-3 (pairs 0,1), half 1 = heads 4-7 (pairs 2,3);
    # half 1 production overlaps pair-0 attention. tt pairs alternate
    # psum banks so consecutive matmuls never accumulate into the same
    # bank back-to-back
    v_all = [po_v.tile([128, VW], F16, tag=f"v{tt}", name=f"v{tt}")
             for tt in range(NTT)]

    def v_units(half, split=1):
        cs = slice(half * 260, half * 260 + 260)
        units = []
        for tt0 in range(0, NTT, 2):
            stt = {}

            def part(tt0=tt0, cs=cs, half=half, stt=stt, c0=0, c1=CCH,
                     fin=True):
                if c0 == 0:
                    stt["ps0"] = pp_qk.tile([128, 260], F32, tag="qk",
                                            name="ps0")
                    stt["ps1"] = pp_qk.tile([128, 260], F32, tag="qk",
                                            name="ps1")
                ps0, ps1 = stt["ps0"], stt["ps1"]
                for c in range(c0, c1):
                    nc.tensor.matmul(
                        ps0[:], xT[c][:, tt0 * 128:(tt0 + 1) * 128],
                        wv_sb[half][c][:], start=(c == 0),
                        stop=(c == CCH - 1))
                    nc.tensor.matmul(
                        ps1[:], xT[c][:, (tt0 + 1) * 128:(tt0 + 2) * 128],
                        wv_sb[half][c][:], start=(c == 0),
                        stop=(c == CCH - 1))
                if fin:
                    nc.vector.tensor_add(v_all[tt0][:, cs], ps0[:],
                                         bva_bc[:, cs])
                    nc.vector.tensor_add(v_all[tt0 + 1][:, cs], ps1[:],
                                         bva_bc[:, cs])

            if split == 1:
                units.append(part)
            else:
                from functools import partial
                units.append(partial(part, c0=0, c1=4, fin=False))
                units.append(partial(part, c0=4, c1=CCH, fin=True))
        return units

    # ---- per head pair: qkv -> attention -> partial proj ----
    # Emitted as interleaved work units so the PE instruction stream mixes
    # next-pair qkv (and couple proj) matmuls between attention groups --
    # engines are in-order, so a blocked exp-wait would otherwise stall
    # ready qkv work behind it.

    def prep_qkv(pair):
        psl = slice(pair * 128, (pair + 1) * 128)
        wqk_c = []
        for c in range(CCH):
            wt = po_wqk.tile([128, 256], F16, tag="wqk", name="wt")
            nc.sync.dma_start(out=wt[:, 0:128],
                              in_=wq[c * 128:(c + 1) * 128, psl])
            nc.sync.dma_start(out=wt[:, 128:256],
                              in_=wk[c * 128:(c + 1) * 128, psl])
            wqk_c.append(wt)
        bq_sb = po_bias.tile([128, 1], F32, tag=f"bq{pair}", name=f"bq{pair}")
        nc.sync.dma_start(out=bq_sb[:], in_=bq[psl, :])
        bk_sb = po_bias.tile([128, 1], F32, tag=f"bk{pair}", name=f"bk{pair}")
        nc.sync.dma_start(out=bk_sb[:], in_=bk[psl, :])
        qT = po_qkt.tile([128, T], F16, tag="qT", name="qT")
        kT = po_qkt.tile([128, T], F16, tag="kT", name="kT")
        return dict(wqk=wqk_c, bq=bq_sb, bk=bk_sb, qT=qT, kT=kT)

    def qkv_units(st8, split=1):
        # split=2 yields two micro-closures per t-block (for weaving
        # between attention steps); psum tile lifetime spans the pair,
        # so micros of one t-block must stay adjacent in their stream
        units = []
        for tb in range(NTB):
            tsl = slice(tb * 512, (tb + 1) * 512)
            stt = {}

            def half(tb=tb, tsl=tsl, stt=stt, c0=0, c1=CCH, fin=True):
                if c0 == 0:
                    stt["psq"] = pp_qk.tile([128, 512], F32, tag="qk",
                                            name="psq")
                    stt["psk"] = pp_qk.tile([128, 512], F32, tag="qk",
                                            name="psk")
                psq, psk = stt["psq"], stt["psk"]
                # q/k matmuls interleaved so consecutive matmuls target
                # alternating psum banks
                for c in range(c0, c1):
                    nc.tensor.matmul(psq[:], st8["wqk"][c][:, 0:128],
                                     xT[c][:, tsl],
                                     start=(c == 0), stop=(c == CCH - 1))
                    nc.tensor.matmul(psk[:], st8["wqk"][c][:, 128:256],
                                     xT[c][:, tsl],
                                     start=(c == 0), stop=(c == CCH - 1))
                if fin:
                    # psum*1/sqrt(D) + bq/sqrt(D)  (bq pre-scaled on host)
                    nc.vector.tensor_scalar(
                        out=st8["qT"][:, tsl], in0=psq[:], scalar1=0.125,
                        scalar2=st8["bq"][:], op0=MULT, op1=ADD)
                    nc.vector.tensor_scalar(
                        out=st8["kT"][:, tsl], in0=psk[:],
                        scalar1=st8["bk"][:], scalar2=None, op0=ADD)

            if split == 1:
                units.append(half)
            else:
                from functools import partial
                units.append(partial(half, c0=0, c1=4, fin=False))
                units.append(partial(half, c0=4, c1=CCH, fin=True))
        return units

    def attn_micros(st8, yt):
        # One q-block unit computes BOTH heads of the pair: the two
        # score matmuls contract over disjoint 64-partition halves
        # (rows 0-63 = head hl0, rows 64-127 = head hl1) so the PE runs
        # them CONCURRENTLY in separate row groups. One st tile packs
        # [hl0 scores | hl1 scores] so a single wide exp covers both.
        # Returns kt-granular micro-closures (for weaving fillers into
        # the exp-latency slack) plus per-unit end indices for gating.
        qT, kT = st8["qT"], st8["kT"]
        micros = []
        ends = []
        for qb in range(NQB):
            nkt = 4 * qb + 4
            ustate = {}

            def emit_scores(kt, st, qb=qb):
                j = kt - 4 * qb
                off = 128 * j if j > 0 else 0
                diag = j >= 0
                ktw = slice(kt * 128, (kt + 1) * 128)
                qw = slice(qb * 512 + off, (qb + 1) * 512)
                nc.tensor.matmul(st[:, off:512], kT[0:64, ktw],
                                 qT[0:64, qw], start=True, stop=not diag)
                if diag:
                    # accumulate the causal mask (and, for hl1 below, the
                    # garbage strip) on the PE: identity stationary x
                    # constant emask moving
                    nc.tensor.matmul(st[:, off:off + 128], ident[:],
                                     emask_sb[:, 384:512],
                                     start=False, stop=True)
                nc.tensor.matmul(st[:, 512 + off:1024], kT[64:128, ktw],
                                 qT[64:128, qw], start=True, stop=not diag)
                if diag:
                    nc.tensor.matmul(st[:, 512:512 + off + 128], ident[:],
                                     emask_sb[:, 384 - off:512],
                                     start=False, stop=True)
                return off

            def emit_exp_pv(kt, st, off, qb=qb, nkt=nkt, ustate=ustate):
                pair = st8["pair"]
                vslA = slice((pair * 2) * 65, (pair * 2) * 65 + 65)
                vslB = slice((pair * 2 + 1) * 65, (pair * 2 + 1) * 65 + 65)
                et = po_exp.tile([128, 1024], F16, tag="expT", name="et")
                nc.scalar.activation(
                    et[:, off:1024], st[:, off:1024],
                    mybir.ActivationFunctionType.Exp)
                nc.tensor.matmul(ustate["pvA"][0:65, off:512],
                                 v_all[kt][:, vslA], et[:, off:512],
                                 start=(kt == 0), stop=(kt == nkt - 1))
                nc.tensor.matmul(ustate["pvB"][0:65, off:512],
                                 v_all[kt][:, vslB], et[:, 512 + off:1024],
                                 start=(kt == 0), stop=(kt == nkt - 1))

            def step(kt, qb=qb, nkt=nkt, ustate=ustate, es=emit_scores,
                     ep=emit_exp_pv):
                if kt == 0:
                    ustate["pvA"] = pp_pv.tile([128, 512], F32, tag="pvA",
                                               name="pvA")
                    ustate["pvB"] = pp_pv.tile([128, 512], F32, tag="pvB",
                                               name="pvB")
                if kt < nkt:
                    st = pp_s.tile([128, 1024], F32, tag="s", name="st")
                    off = es(kt, st, qb=qb)
                    pend = ustate.get("pend")
                    if pend is not None:
                        ep(*pend)
                    ustate["pend"] = (kt, st, off)
                else:
                    ep(*ustate["pend"])
                    ustate["pend"] = None

            def tail(hl, qb=qb, ustate=ustate):
                # den row 64 -> DRAM bounce broadcast -> recip -> mul
                qsl = slice(qb * 512, (qb + 1) * 512)
                pv = ustate["pvA"] if hl == 0 else ustate["pvB"]
                den = po_den.tile([128, 512], F32, tag="den", name="den")
                nc.vector.tensor_copy(den[64:65, :], pv[64:65, :])
                dscr = po_dram.tile([1, 512], F32, tag="dscr", name="dscr")
                # gpsimd SWDGE queue: independent of the bulk-load/store
                # HWDGE rings, so these latency-critical hops never queue
                # behind 256KB transfers
                nc.gpsimd.dma_start(out=dscr[:], in_=den[64:65, :])
                rec = po_rec.tile([128, 512], F32, tag="recip", name="rec")
                nc.gpsimd.dma_start(out=rec[0:64, :], in_=bass.AP(
                    tensor=dscr.tensor, offset=dscr[:].offset,
                    ap=[[0, 64]] + [list(a) for a in dscr[:].ap[1:]]))
                nc.vector.reciprocal_approx_fast(rec[0:64, :], rec[0:64, :])
                if hl == 0:
                    nc.vector.tensor_mul(yt[0:64, qsl], pv[0:64, :],
                                         rec[0:64, :])
                else:
                    # engines can't cross partitions; bounce via DMA
                    ytmp = po_ytmp.tile([128, 512], F16, tag="ytmp",
                                        name="ytmp")
                    nc.vector.tensor_mul(ytmp[0:64, :], pv[0:64, :],
                                         rec[0:64, :])
                    nc.gpsimd.dma_start(out=yt[64:128, qsl],
                                        in_=ytmp[0:64, :])

            from functools import partial
            for kt in range(nkt):
                micros.append(partial(step, kt))
            def last(ustate=ustate, step=step, tail=tail, nkt=nkt):
                step(nkt)
                tail(0)
            micros.append(last)
            micros.append(partial(tail, 1))
            ends.append(len(micros) - 1)
        return micros, ends

    def prep_proj(couple):
        wp_sb = []
        for pq in range(2):
            for cb in range(2):
                prow = (couple * 2 + pq) * 128
                wt = po_wp.tile([128, 512], F16, tag="wp", name="wpt")
                nc.sync.dma_start(
                    out=wt[:],
                    in_=wp[prow:prow + 128, cb * 512:(cb + 1) * 512])
                wp_sb.append(wt)
        return wp_sb

    def proj_micros(couple, wp_sb, yts, tts):
        out_p = out_ab[couple]
        micros = []
        for tt in tts:
            def micro(tt=tt):
                ot = po_misc.tile([128, C], F16, tag="misc", name="ot")
                ps0 = pp_qk.tile([128, 512], F32, tag="qk", name="pp0")
                ps1 = pp_qk.tile([128, 512], F32, tag="qk", name="pp1")
                pss = (ps0, ps1)
                # pq outer: the y stationary is reused across the two
                # cb matmuls, which alternate psum banks
                for pq in range(2):
                    for cb in range(2):
                        nc.tensor.matmul(
                            pss[cb][:],
                            yts[pq][:, tt * 128:(tt + 1) * 128],
                            wp_sb[pq * 2 + cb][:],
                            start=(pq == 0), stop=(pq == 1))
                for cb in range(2):
                    nc.vector.tensor_copy(
                        ot[:, cb * 512:(cb + 1) * 512], pss[cb][:])
                nc.sync.dma_start(
                    out=out_p[tt * 128:(tt + 1) * 128, :], in_=ot[:])
            micros.append(micro)
        return micros

    def round_robin(*streams):
        streams = [list(s) for s in streams if s]
        while any(streams):
            for s in streams:
                if s:
                    s.pop(0)()

    def weave(primary, fillers, gates=None):
        # Spread filler micro-closures evenly between primary ones;
        # gates[i] = index into primary that must already be emitted
        # before fillers[i] may run.
        nf, npr = len(fillers), len(primary)
        r = nf / npr if npr else 0.0
        acc, fi = 0.0, 0
        for pi, u in enumerate(primary):
            u()
            acc += r
            while (fi < nf and acc >= 1.0
                   and (gates is None or gates[fi] <= pi)):
                fillers[fi]()
                fi += 1
                acc -= 1.0
        while fi < nf:
            fillers[fi]()
            fi += 1

    pair_state = []
    yts = []
    st0 = prep_qkv(0)
    st0["pair"] = 0
    pair_state.append(st0)
    load_xt()
    # pair-0 qkv races ahead of v-half0 so pair-0 attention (which only
    # needs half-0 v) starts as early as possible; v-half1 (for pairs
    # 2,3) overlaps pair-0 attention
    round_robin(qkv_units(st0), v_units(0))
    wp_sb0 = wp_sb1 = None
    for p in range(NPAIR):
        yt = po_yt.tile([128, T], F16, tag="yT", name="yt")
        yts.append(yt)
        am, ends = attn_micros(pair_state[p], yt)
        fillers = []
        gates = []
        if p + 1 < NPAIR:
            stn = prep_qkv(p + 1)
            stn["pair"] = p + 1
            pair_state.append(stn)
            qm = qkv_units(stn, split=2)
            fillers += qm
            gates += [-1] * len(qm)
        if p == 0:
            vm = v_units(1, split=2)
            fillers += vm
            gates += [-1] * len(vm)
        if p == 1:
            # couple-0 proj: first half here, gated on the attn-1
            # q-blocks that complete the yts[1] columns it reads
            wp_sb0 = prep_proj(0)
            pm = proj_micros(0, wp_sb0, yts[0:2], range(0, 8))
            fillers += pm
            gates += [ends[tt // 4] for tt in range(0, 8)]
        if p == 2:
            pm = proj_micros(0, wp_sb0, yts[0:2], range(8, 16))
            fillers += pm
            gates += [-1] * len(pm)
        if p == 3:
            wp_sb1 = prep_proj(1)
            pm = proj_micros(1, wp_sb1, yts[2:4], range(0, 16))
            fillers += pm
            gates += [ends[tt // 4] for tt in range(0, 16)]
        weave(am, fillers, gates)

    ctx.close()


_CACHE = {}


def _build():
    if "nc" in _CACHE:
        return _CACHE["nc"]
    nc = bacc.Bacc("TRN2", target_bir_lowering=False, debug=False,
                   enable_asserts=True, num_devices=N_CORES)
    aps = {
        "x": nc.dram_tensor("x", [C, T], F16, kind="ExternalInput").ap(),
        "wq": nc.dram_tensor("wq", [C, F], F16, kind="ExternalInput").ap(),
        "wk": nc.dram_tensor("wk", [C, F], F16, kind="ExternalInput").ap(),
        "wva": nc.dram_tensor("wva", [C, VW], F16, kind="ExternalInput").ap(),
        "bq": nc.dram_tensor("bq", [F, 1], F32, kind="ExternalInput").ap(),
        "bk": nc.dram_tensor("bk", [F, 1], F32, kind="ExternalInput").ap(),
        "bva2": nc.dram_tensor("bva2", [1, VW], F32, kind="ExternalInput").ap(),
        "wp": nc.dram_tensor("wp", [F, C], F16, kind="ExternalInput").ap(),
        "emask": nc.dram_tensor("emask", [128, 512], F16,
                                kind="ExternalInput").ap(),
        "identin": nc.dram_tensor("identin", [128, 128], F16,
                                  kind="ExternalInput").ap(),
        "out_pa": nc.dram_tensor("out_pa", [T, C], F16,
                                 kind="ExternalOutput").ap(),
        "out_pb": nc.dram_tensor("out_pb", [T, C], F16,
                                 kind="ExternalOutput").ap(),
    }
    with tile.TileContext(nc) as tc:
        _emit(tc, aps)
    nc.compile()
    _CACHE["nc"] = nc
    return nc


def _make_in_maps(x, Wqkv, bqkv, Wproj):
    x = np.asarray(x, dtype=np.float32)
    Wqkv = np.asarray(Wqkv, dtype=np.float32)
    bqkv = np.asarray(bqkv, dtype=np.float32)
    Wproj = np.asarray(Wproj, dtype=np.float32)

    # emask[p, w] = -6e4 if w < 384 + p else 0 (accumulated onto scores
    # by the PE; -6e4 stays within f16 range and exp() underflows to 0)
    p_idx = np.arange(128)[:, None]
    w_idx = np.arange(512)[None, :]
    emask = np.where(w_idx < 384 + p_idx, -6e4, 0.0).astype(np.float16)

    in_maps = []
    for core in range(N_CORES):
        b, g = divmod(core, 2)
        q0, k0, v0 = 512 * g, C + 512 * g, 2 * C + 512 * g
        wva = np.zeros((C, VW), dtype=np.float32)
        bva = np.zeros((1, VW), dtype=np.float32)
        for h in range(NH):
            src = v0 + D * h
            dst = 65 * h
            # per-head layout [v(64), one]
            wva[:, dst:dst + 64] = Wqkv[:, src:src + 64]
            bva[0, dst:dst + 64] = bqkv[src:src + 64]
            bva[0, dst + 64] = 1.0
        in_maps.append({
            "x": np.ascontiguousarray(x[b].T).astype(np.float16),
            "wq": np.ascontiguousarray(Wqkv[:, q0:q0 + F]).astype(np.float16),
            "wk": np.ascontiguousarray(Wqkv[:, k0:k0 + F]).astype(np.float16),
            "wva": wva.astype(np.float16),
            "bq": np.ascontiguousarray(bqkv[q0:q0 + F].reshape(F, 1) * 0.125),
            "bk": np.ascontiguousarray(bqkv[k0:k0 + F].reshape(F, 1)),
            "bva2": bva,
            "wp": np.ascontiguousarray(Wproj[512 * g:512 * g + F, :]).astype(np.float16),
            "emask": emask,
            "identin": np.eye(128, dtype=np.float16),
        })
    return in_maps


def run_sharded(x, Wqkv, bqkv, Wproj, bproj, trace=False):
    nc = _build()
    in_maps = _make_in_maps(x, Wqkv, bqkv, Wproj)
    res = run_bass_kernel_spmd(nc, in_maps, core_ids=list(range(N_CORES)),
                               trace=trace)
    bproj = np.asarray(bproj, dtype=np.float32)
    out = np.empty((B, T, C), dtype=np.float32)
    for b in range(B):
        acc = bproj[None, :].astype(np.float32).repeat(T, axis=0)
        for core in (2 * b, 2 * b + 1):
            acc = (acc + res.results[core]["out_pa"].astype(np.float32)
                   + res.results[core]["out_pb"].astype(np.float32))
        out[b] = acc
    return out, res


def kernel(x, Wqkv, bqkv, Wproj, bproj):
    out, _ = run_sharded(x, Wqkv, bqkv, Wproj, bproj, trace=False)
    return out



# revision 43
# speedup vs baseline: 1.0243x; 1.0150x over previous
"""Causal self-attention (B=4, T=2048, C=1024, H=16, Dh=64) on 8 trn2 NeuronCores.

Sharding: core i <-> (batch b = i//2, head-group g = i%2). Each core computes
8 heads of one batch end-to-end (qkv slice, causal attention, partial output
projection); the host sums the head-group/pair-couple partials per batch and
adds bproj. No device collectives.

x arrives host-pretransposed as xT[C, T] (fp16), so qkv matmuls stream it
directly with the contraction dim on partitions -- no on-device transposes.
Attention uses the transposed-scores layout sT[tk, tq]: softmax denominators
come out of the PV matmul via an extra ones column interleaved into Wv, and
are broadcast across partitions with a partition-step-0 SBUF->SBUF DMA.
Partial projection outputs are written fp16 and summed on the host.
"""

import numpy as np

import concourse.bass as bass
import concourse.tile as tile
from concourse import bacc, mybir
from concourse.bass_utils import run_bass_kernel_spmd

F32 = mybir.dt.float32
F32R = mybir.dt.float32r
F16 = mybir.dt.float16

N_CORES = 8
B, T, C = 4, 2048, 1024
NH_TOT, D = 16, 64
F = 512            # features per core (8 heads)
NH = 8             # local heads
NPAIR = 4          # head pairs (128 feats each)
CCH = C // 128     # 8 contraction chunks
NTT = T // 128     # 16 t tiles
NTB = T // 512     # 4 t blocks (qkv production)
NQB = T // 512     # 4 q blocks (attention)
VW = NH * (D + 1)  # 520: augmented v width
ADD = mybir.AluOpType.add
MULT = mybir.AluOpType.mult


def _emit(tc, aps):
    from contextlib import ExitStack
    nc = tc.nc
    x, wq, wk, wva, bq, bk, wp = (
        aps["x"], aps["wq"], aps["wk"], aps["wva"], aps["bq"], aps["bk"],
        aps["wp"])
    out_ab = [aps["out_pa"], aps["out_pb"]]

    # ---- pools (all coexist; ~210KB/partition total) ----
    ctx = ExitStack()
    pp_qk = ctx.enter_context(tc.tile_pool(name="ps_qk", bufs=2, space="PSUM"))
    pp_s = ctx.enter_context(tc.tile_pool(name="ps_s", bufs=2, space="PSUM"))
    pp_pv = ctx.enter_context(tc.tile_pool(name="ps_pv", bufs=1, space="PSUM"))
    po_v = ctx.enter_context(tc.tile_pool(name="v_all", bufs=1))
    po_mask = ctx.enter_context(tc.tile_pool(name="mask", bufs=1))
    po_wv = ctx.enter_context(tc.tile_pool(name="wv", bufs=16))
    po_qkt = ctx.enter_context(tc.tile_pool(name="qkT", bufs=2))
    po_bias = ctx.enter_context(tc.tile_pool(name="bias", bufs=1))
    po_misc = ctx.enter_context(tc.tile_pool(name="misc", bufs=3))
    po_xt = ctx.enter_context(tc.tile_pool(name="xT", bufs=1))
    po_wqk = ctx.enter_context(tc.tile_pool(name="wqk", bufs=2))
    po_yt = ctx.enter_context(tc.tile_pool(name="yT", bufs=4))
    po_exp = ctx.enter_context(tc.tile_pool(name="expT", bufs=4))
    po_rec = ctx.enter_context(tc.tile_pool(name="recip", bufs=3))
    po_den = ctx.enter_context(tc.tile_pool(name="den", bufs=2))
    po_ytmp = ctx.enter_context(tc.tile_pool(name="ytmp", bufs=2))
    po_wp = ctx.enter_context(tc.tile_pool(name="wp", bufs=4))
    po_dram = ctx.enter_context(tc.tile_pool(name="dram_scr", bufs=4,
                                             space="DRAM"))

    # emask[p, w] = -6e4 if w < 384 + p else 0: sliced at [384-off, 512)
    # it masks the diagonal 128-col triangle plus the garbage strip, and
    # is ACCUMULATED onto scores by the PE itself (identity stationary)
    # so the softmax chain never hops through the vector engine
    emask_sb = po_mask.tile([128, 512], F16, tag="emask")
    nc.scalar.dma_start(out=emask_sb[:], in_=aps["emask"][:])
    ident = po_bias.tile([128, 128], F16, tag="ident")
    nc.scalar.dma_start(out=ident[:], in_=aps["identin"][:])
    # bva broadcast to all 128 partitions straight from DRAM
    bva_bc = po_bias.tile([128, VW], F32, tag="bva_bc")
    bva2 = aps["bva2"]
    nc.scalar.dma_start(out=bva_bc[:], in_=bass.AP(
        tensor=bva2.tensor, offset=bva2.offset,
        ap=[[0, 128]] + [list(a) for a in bva2.ap[1:]]))

    # ---- phase 0: small weights first (pair-0 qkv + wv), then the 4MB
    # xT so the first qkv/v matmuls unblock as early as possible; wv on
    # the scalar HWDGE ring, xT on the sync ring -- rings run in parallel
    xT = [po_xt.tile([128, T], F16, tag=f"xT{c}", name=f"xT{c}")
          for c in range(CCH)]
    wv_sb = [[None] * CCH, [None] * CCH]
    for c in range(CCH):
        for half in range(2):
            cs = slice(half * 260, half * 260 + 260)
            wt = po_wv.tile([128, 260], F16, tag="wv")
            nc.scalar.dma_start(out=wt[:], in_=wva[c * 128:(c + 1) * 128, cs])
            wv_sb[half][c] = wt

    def load_xt():
        for c in range(CCH):
            nc.sync.dma_start(out=xT[c][:], in_=x[c * 128:(c + 1) * 128, :])

    # ---- phase 0b: v (augmented with ones columns, all 8 heads) ----
    # half 0 = heads 0-3 (pairs 0,1), half 1 = heads 4-7 (pairs 2,3);
    # half 1 production overlaps pair-0 attention. tt pairs alternate
    # psum banks so consecutive matmuls never accumulate into the same
    # bank back-to-back
    v_all = [po_v.tile([128, VW], F16, tag=f"v{tt}", name=f"v{tt}")
             for tt in range(NTT)]

    def v_units(half, split=1):
        cs = slice(half * 260, half * 260 + 260)
        units = []
        for tt0 in range(0, NTT, 2):
            stt = {}

            def part(tt0=tt0, cs=cs, half=half, stt=stt, c0=0, c1=CCH,
                     fin=True):
                if c0 == 0:
                    stt["ps0"] = pp_qk.tile([128, 260], F32, tag="qk",
                                            name="ps0")
                    stt["ps1"] = pp_qk.tile([128, 260], F32, tag="qk",
                                            name="ps1")
                ps0, ps1 = stt["ps0"], stt["ps1"]
                for c in range(c0, c1):
                    nc.tensor.matmul(
                        ps0[:], xT[c][:, tt0 * 128:(tt0 + 1) * 128],
                        wv_sb[half][c][:], start=(c == 0),
                        stop=(c == CCH - 1))
                    nc.tensor.matmul(
                        ps1[:], xT[c][:, (tt0 + 1) * 128:(tt0 + 2) * 128],
                        wv_sb[half][c][:], start=(c == 0),
                        stop=(c == CCH - 1))
                if fin:
                    nc.vector.tensor_add(v_all[tt0][:, cs], ps0[:],
                                         bva_bc[:, cs])
                    nc.vector.tensor_add(v_all[tt0 + 1][:, cs], ps1[:],
                                         bva_bc[:, cs])

            if split == 1:
                units.append(part)
            else:
                from functools import partial
                units.append(partial(part, c0=0, c1=4, fin=False))
                units.append(partial(part, c0=4, c1=CCH, fin=True))
        return units

    # ---- per head pair: qkv -> attention -> partial proj ----
    # Emitted as interleaved work units so the PE instruction stream mixes
    # next-pair qkv (and couple proj) matmuls between attention groups --
    # engines are in-order, so a blocked exp-wait would otherwise stall
    # ready qkv work behind it.

    def prep_qkv(pair):
        psl = slice(pair * 128, (pair + 1) * 128)
        # one [128, 8*256] tile: chunk c at cols [256c, 256c+128) = wq,
        # [256c+128, 256(c+1)) = wk. Loaded with TWO strided DMAs (the
        # 16 little per-chunk DMAs serialize ~1us each on the ring)
        wqk_all = po_wqk.tile([128, 256 * CCH], F16, tag="wqk",
                              name="wqk_all")
        dap = wqk_all[:]
        dstep = dap.ap[1][0]
        for which, w in ((0, wq), (1, wk)):
            src = w[:, psl]
            sstep_r, sstep_e = src.ap[0][0], src.ap[1][0]
            nc.sync.dma_start(
                out=bass.AP(
                    tensor=dap.tensor,
                    offset=dap.offset + which * 128 * dstep,
                    ap=[list(dap.ap[0]),
                        [256 * dstep, CCH], [dstep, 128]]),
                in_=bass.AP(
                    tensor=src.tensor, offset=src.offset,
                    ap=[[sstep_r, 128], [sstep_r * 128, CCH],
                        [sstep_e, 128]]))
        wqk_c = [wqk_all[:, 256 * c:256 * (c + 1)] for c in range(CCH)]
        bq_sb = po_bias.tile([128, 1], F32, tag=f"bq{pair}", name=f"bq{pair}")
        nc.sync.dma_start(out=bq_sb[:], in_=bq[psl, :])
        bk_sb = po_bias.tile([128, 1], F32, tag=f"bk{pair}", name=f"bk{pair}")
        nc.sync.dma_start(out=bk_sb[:], in_=bk[psl, :])
        qT = po_qkt.tile([128, T], F16, tag="qT", name="qT")
        kT = po_qkt.tile([128, T], F16, tag="kT", name="kT")
        return dict(wqk=wqk_c, bq=bq_sb, bk=bk_sb, qT=qT, kT=kT)

    def qkv_units(st8, split=1):
        # split=2 yields two micro-closures per t-block (for weaving
        # between attention steps); psum tile lifetime spans the pair,
        # so micros of one t-block must stay adjacent in their stream
        units = []
        for tb in range(NTB):
            tsl = slice(tb * 512, (tb + 1) * 512)
            stt = {}

            def half(tb=tb, tsl=tsl, stt=stt, c0=0, c1=CCH, fin=True):
                if c0 == 0:
                    stt["psq"] = pp_qk.tile([128, 512], F32, tag="qk",
                                            name="psq")
                    stt["psk"] = pp_qk.tile([128, 512], F32, tag="qk",
                                            name="psk")
                psq, psk = stt["psq"], stt["psk"]
                # q/k matmuls interleaved so consecutive matmuls target
                # alternating psum banks
                for c in range(c0, c1):
                    nc.tensor.matmul(psq[:], st8["wqk"][c][:, 0:128],
                                     xT[c][:, tsl],
                                     start=(c == 0), stop=(c == CCH - 1))
                    nc.tensor.matmul(psk[:], st8["wqk"][c][:, 128:256],
                                     xT[c][:, tsl],
                                     start=(c == 0), stop=(c == CCH - 1))
                if fin:
                    # psum*1/sqrt(D) + bq/sqrt(D)  (bq pre-scaled on host)
                    nc.vector.tensor_scalar(
                        out=st8["qT"][:, tsl], in0=psq[:], scalar1=0.125,
                        scalar2=st8["bq"][:], op0=MULT, op1=ADD)
                    nc.vector.tensor_scalar(
                        out=st8["kT"][:, tsl], in0=psk[:],
                        scalar1=st8["bk"][:], scalar2=None, op0=ADD)

            if split == 1:
                units.append(half)
            else:
                from functools import partial
                units.append(partial(half, c0=0, c1=4, fin=False))
                units.append(partial(half, c0=4, c1=CCH, fin=True))
        return units

    def attn_micros(st8, yt):
        # One q-block unit computes BOTH heads of the pair: the two
        # score matmuls contract over disjoint 64-partition halves
        # (rows 0-63 = head hl0, rows 64-127 = head hl1) so the PE runs
        # them CONCURRENTLY in separate row groups. One st tile packs
        # [hl0 scores | hl1 scores] so a single wide exp covers both.
        # Returns kt-granular micro-closures (for weaving fillers into
        # the exp-latency slack) plus per-unit end indices for gating.
        qT, kT = st8["qT"], st8["kT"]
        micros = []
        ends = []
        for qb in range(NQB):
            nkt = 4 * qb + 4
            ustate = {}

            def emit_scores(kt, st, qb=qb):
                j = kt - 4 * qb
                off = 128 * j if j > 0 else 0
                diag = j >= 0
                ktw = slice(kt * 128, (kt + 1) * 128)
                qw = slice(qb * 512 + off, (qb + 1) * 512)
                nc.tensor.matmul(st[:, off:512], kT[0:64, ktw],
                                 qT[0:64, qw], start=True, stop=not diag)
                if diag:
                    # accumulate the causal mask (and, for hl1 below, the
                    # garbage strip) on the PE: identity stationary x
                    # constant emask moving
                    nc.tensor.matmul(st[:, off:off + 128], ident[:],
                                     emask_sb[:, 384:512],
                                     start=False, stop=True)
                nc.tensor.matmul(st[:, 512 + off:1024], kT[64:128, ktw],
                                 qT[64:128, qw], start=True, stop=not diag)
                if diag:
                    nc.tensor.matmul(st[:, 512:512 + off + 128], ident[:],
                                     emask_sb[:, 384 - off:512],
                                     start=False, stop=True)
                return off

            def emit_exp_pv(kt, st, off, qb=qb, nkt=nkt, ustate=ustate):
                pair = st8["pair"]
                vslA = slice((pair * 2) * 65, (pair * 2) * 65 + 65)
                vslB = slice((pair * 2 + 1) * 65, (pair * 2 + 1) * 65 + 65)
                et = po_exp.tile([128, 1024], F16, tag="expT", name="et")
                nc.scalar.activation(
                    et[:, off:1024], st[:, off:1024],
                    mybir.ActivationFunctionType.Exp)
                nc.tensor.matmul(ustate["pvA"][0:65, off:512],
                                 v_all[kt][:, vslA], et[:, off:512],
                                 start=(kt == 0), stop=(kt == nkt - 1))
                nc.tensor.matmul(ustate["pvB"][0:65, off:512],
                                 v_all[kt][:, vslB], et[:, 512 + off:1024],
                                 start=(kt == 0), stop=(kt == nkt - 1))

            def step(kt, qb=qb, nkt=nkt, ustate=ustate, es=emit_scores,
                     ep=emit_exp_pv):
                if kt == 0:
                    ustate["pvA"] = pp_pv.tile([128, 512], F32, tag="pvA",
                                               name="pvA")
                    ustate["pvB"] = pp_pv.tile([128, 512], F32, tag="pvB",
                                               name="pvB")
                if kt < nkt:
                    st = pp_s.tile([128, 1024], F32, tag="s", name="st")
                    off = es(kt, st, qb=qb)
                    pend = ustate.get("pend")
                    if pend is not None:
                        ep(*pend)
                    ustate["pend"] = (kt, st, off)
                else:
                    ep(*ustate["pend"])
                    ustate["pend"] = None

            def tail(hl, qb=qb, ustate=ustate):
                # den row 64 -> DRAM bounce broadcast -> recip -> mul
                qsl = slice(qb * 512, (qb + 1) * 512)
                pv = ustate["pvA"] if hl == 0 else ustate["pvB"]
                den = po_den.tile([128, 512], F32, tag="den", name="den")
                # scalar engine: keeps the copy off the busier vector
                # queue (chain: copy -> DMA bounce -> recip -> mul gates
                # the pv-bank reuse of the next q-block)
                nc.scalar.copy(den[64:65, :], pv[64:65, :])
                dscr = po_dram.tile([1, 512], F32, tag="dscr", name="dscr")
                # gpsimd SWDGE queue: independent of the bulk-load/store
                # HWDGE rings, so these latency-critical hops never queue
                # behind 256KB transfers
                nc.gpsimd.dma_start(out=dscr[:], in_=den[64:65, :])
                rec = po_rec.tile([128, 512], F32, tag="recip", name="rec")
                nc.gpsimd.dma_start(out=rec[0:64, :], in_=bass.AP(
                    tensor=dscr.tensor, offset=dscr[:].offset,
                    ap=[[0, 64]] + [list(a) for a in dscr[:].ap[1:]]))
                nc.vector.reciprocal_approx_fast(rec[0:64, :], rec[0:64, :])
                if hl == 0:
                    nc.vector.tensor_mul(yt[0:64, qsl], pv[0:64, :],
                                         rec[0:64, :])
                else:
                    # engines can't cross partitions; bounce via DMA
                    ytmp = po_ytmp.tile([128, 512], F16, tag="ytmp",
                                        name="ytmp")
                    nc.vector.tensor_mul(ytmp[0:64, :], pv[0:64, :],
                                         rec[0:64, :])
                    nc.gpsimd.dma_start(out=yt[64:128, qsl],
                                        in_=ytmp[0:64, :])

            from functools import partial
            for kt in range(nkt):
                micros.append(partial(step, kt))
            def last(ustate=ustate, step=step, tail=tail, nkt=nkt):
                step(nkt)
                tail(0)
            micros.append(last)
            micros.append(partial(tail, 1))
            ends.append(len(micros) - 1)
        return micros, ends

    def prep_proj(couple):
        wp_sb = []
        for pq in range(2):
            for cb in range(2):
                prow = (couple * 2 + pq) * 128
                wt = po_wp.tile([128, 512], F16, tag="wp", name="wpt")
                nc.sync.dma_start(
                    out=wt[:],
                    in_=wp[prow:prow + 128, cb * 512:(cb + 1) * 512])
                wp_sb.append(wt)
        return wp_sb

    def proj_micros(couple, wp_sb, yts, tts):
        out_p = out_ab[couple]
        micros = []
        for tt in tts:
            def micro(tt=tt):
                ot = po_misc.tile([128, C], F16, tag="misc", name="ot")
                ps0 = pp_qk.tile([128, 512], F32, tag="qk", name="pp0")
                ps1 = pp_qk.tile([128, 512], F32, tag="qk", name="pp1")
                pss = (ps0, ps1)
                # pq outer: the y stationary is reused across the two
                # cb matmuls, which alternate psum banks
                for pq in range(2):
                    for cb in range(2):
                        nc.tensor.matmul(
                            pss[cb][:],
                            yts[pq][:, tt * 128:(tt + 1) * 128],
                            wp_sb[pq * 2 + cb][:],
                            start=(pq == 0), stop=(pq == 1))
                for cb in range(2):
                    nc.vector.tensor_copy(
                        ot[:, cb * 512:(cb + 1) * 512], pss[cb][:])
                nc.sync.dma_start(
                    out=out_p[tt * 128:(tt + 1) * 128, :], in_=ot[:])
            micros.append(micro)
        return micros

    def round_robin(*streams):
        streams = [list(s) for s in streams if s]
        while any(streams):
            for s in streams:
                if s:
                    s.pop(0)()

    def weave(primary, fillers, gates=None, boost=()):
        # Spread filler micro-closures evenly between primary ones;
        # gates[i] = index into primary that must already be emitted
        # before fillers[i] may run. boost = primary indices (attention
        # unit tails) after which extra fillers are popped to cover the
        # den-bounce chain that gates pv-bank reuse.
        nf, npr = len(fillers), len(primary)
        r = nf / npr if npr else 0.0
        acc, fi = 0.0, 0
        boost = set(boost)
        for pi, u in enumerate(primary):
            u()
            acc += r
            if pi in boost:
                acc += 2.0
            while (fi < nf and acc >= 1.0
                   and (gates is None or gates[fi] <= pi)):
                fillers[fi]()
                fi += 1
                acc -= 1.0
        while fi < nf:
            fillers[fi]()
            fi += 1

    pair_state = []
    yts = []
    st0 = prep_qkv(0)
    st0["pair"] = 0
    pair_state.append(st0)
    load_xt()
    # pair-0 qkv races ahead of v-half0 so pair-0 attention (which only
    # needs half-0 v) starts as early as possible; v-half1 (for pairs
    # 2,3) overlaps pair-0 attention
    round_robin(qkv_units(st0), v_units(0))
    # p0: attn0 + (qkv1, v-half1) fillers
    yt0 = po_yt.tile([128, T], F16, tag="yT", name="yt0")
    yts.append(yt0)
    am0, ends0 = attn_micros(pair_state[0], yt0)
    st1 = prep_qkv(1)
    st1["pair"] = 1
    pair_state.append(st1)
    f0 = qkv_units(st1, split=2) + v_units(1, split=2)
    weave(am0, f0, [-1] * len(f0), boost=ends0)

    # p1: attn1 + (qkv2, first half of couple-0 proj) fillers
    yt1 = po_yt.tile([128, T], F16, tag="yT", name="yt1")
    yts.append(yt1)
    am1, ends1 = attn_micros(pair_state[1], yt1)
    st2 = prep_qkv(2)
    st2["pair"] = 2
    pair_state.append(st2)
    wp_sb0 = prep_proj(0)
    f1 = qkv_units(st2, split=2)
    g1 = [-1] * len(f1)
    f1 += proj_micros(0, wp_sb0, yts[0:2], range(0, 8))
    g1 += [ends1[tt // 4] for tt in range(0, 8)]
    weave(am1, f1, g1, boost=ends1)

    # p2+p3 merged: one weave so fillers flow across the boundary
    yt2 = po_yt.tile([128, T], F16, tag="yT", name="yt2")
    yt3 = po_yt.tile([128, T], F16, tag="yT", name="yt3")
    yts += [yt2, yt3]
    st3 = prep_qkv(3)
    st3["pair"] = 3
    pair_state.append(st3)
    am2, ends2 = attn_micros(pair_state[2], yt2)
    am3, ends3 = attn_micros(pair_state[3], yt3)
    wp_sb1 = prep_proj(1)
    am23 = am2 + am3
    ends23 = list(ends2) + [len(am2) + e for e in ends3]
    f2 = qkv_units(st3, split=2)
    g2 = [-1] * len(f2)
    f2 += proj_micros(0, wp_sb0, yts[0:2], range(8, 16))
    g2 += [-1] * 8
    f2 += proj_micros(1, wp_sb1, yts[2:4], range(0, 16))
    g2 += [len(am2) + ends3[tt // 4] for tt in range(0, 16)]
    weave(am23, f2, g2, boost=ends23)

    ctx.close()


_CACHE = {}


def _build():
    if "nc" in _CACHE:
        return _CACHE["nc"]
    nc = bacc.Bacc("TRN2", target_bir_lowering=False, debug=False,
                   enable_asserts=True, num_devices=N_CORES)
    aps = {
        "x": nc.dram_tensor("x", [C, T], F16, kind="ExternalInput").ap(),
        "wq": nc.dram_tensor("wq", [C, F], F16, kind="ExternalInput").ap(),
        "wk": nc.dram_tensor("wk", [C, F], F16, kind="ExternalInput").ap(),
        "wva": nc.dram_tensor("wva", [C, VW], F16, kind="ExternalInput").ap(),
        "bq": nc.dram_tensor("bq", [F, 1], F32, kind="ExternalInput").ap(),
        "bk": nc.dram_tensor("bk", [F, 1], F32, kind="ExternalInput").ap(),
        "bva2": nc.dram_tensor("bva2", [1, VW], F32, kind="ExternalInput").ap(),
        "wp": nc.dram_tensor("wp", [F, C], F16, kind="ExternalInput").ap(),
        "emask": nc.dram_tensor("emask", [128, 512], F16,
                                kind="ExternalInput").ap(),
        "identin": nc.dram_tensor("identin", [128, 128], F16,
                                  kind="ExternalInput").ap(),
        "out_pa": nc.dram_tensor("out_pa", [T, C], F16,
                                 kind="ExternalOutput").ap(),
        "out_pb": nc.dram_tensor("out_pb", [T, C], F16,
                                 kind="ExternalOutput").ap(),
    }
    with tile.TileContext(nc) as tc:
        _emit(tc, aps)
    nc.compile()
    _CACHE["nc"] = nc
    return nc


def _make_in_maps(x, Wqkv, bqkv, Wproj):
    x = np.asarray(x, dtype=np.float32)
    Wqkv = np.asarray(Wqkv, dtype=np.float32)
    bqkv = np.asarray(bqkv, dtype=np.float32)
    Wproj = np.asarray(Wproj, dtype=np.float32)

    # emask[p, w] = -6e4 if w < 384 + p else 0 (accumulated onto scores
    # by the PE; -6e4 stays within f16 range and exp() underflows to 0)
    p_idx = np.arange(128)[:, None]
    w_idx = np.arange(512)[None, :]
    emask = np.where(w_idx < 384 + p_idx, -6e4, 0.0).astype(np.float16)

    in_maps = []
    for core in range(N_CORES):
        b, g = divmod(core, 2)
        q0, k0, v0 = 512 * g, C + 512 * g, 2 * C + 512 * g
        wva = np.zeros((C, VW), dtype=np.float32)
        bva = np.zeros((1, VW), dtype=np.float32)
        for h in range(NH):
            src = v0 + D * h
            dst = 65 * h
            # per-head layout [v(64), one]
            wva[:, dst:dst + 64] = Wqkv[:, src:src + 64]
            bva[0, dst:dst + 64] = bqkv[src:src + 64]
            bva[0, dst + 64] = 1.0
        in_maps.append({
            "x": np.ascontiguousarray(x[b].T).astype(np.float16),
            "wq": np.ascontiguousarray(Wqkv[:, q0:q0 + F]).astype(np.float16),
            "wk": np.ascontiguousarray(Wqkv[:, k0:k0 + F]).astype(np.float16),
            "wva": wva.astype(np.float16),
            "bq": np.ascontiguousarray(bqkv[q0:q0 + F].reshape(F, 1) * 0.125),
            "bk": np.ascontiguousarray(bqkv[k0:k0 + F].reshape(F, 1)),
            "bva2": bva,
            "wp": np.ascontiguousarray(Wproj[512 * g:512 * g + F, :]).astype(np.float16),
            "emask": emask,
            "identin": np.eye(128, dtype=np.float16),
        })
    return in_maps


def run_sharded(x, Wqkv, bqkv, Wproj, bproj, trace=False):
    nc = _build()
    in_maps = _make_in_maps(x, Wqkv, bqkv, Wproj)
    res = run_bass_kernel_spmd(nc, in_maps, core_ids=list(range(N_CORES)),
                               trace=trace)
    bproj = np.asarray(bproj, dtype=np.float32)
    out = np.empty((B, T, C), dtype=np.float32)
    for b in range(B):
        acc = bproj[None, :].astype(np.float32).repeat(T, axis=0)
        for core in (2 * b, 2 * b + 1):
            acc = (acc + res.results[core]["out_pa"].astype(np.float32)
                   + res.results[core]["out_pb"].astype(np.float32))
        out[b] = acc
    return out, res


def kernel(x, Wqkv, bqkv, Wproj, bproj):
    out, _ = run_sharded(x, Wqkv, bqkv, Wproj, bproj, trace=False)
    return out



# revision 51
# speedup vs baseline: 1.0525x; 1.0275x over previous
"""Causal self-attention (B=4, T=2048, C=1024, H=16, Dh=64) on 8 trn2 NeuronCores.

Sharding: core i <-> (batch b = i//2, head-group g = i%2). Each core computes
8 heads of one batch end-to-end (qkv slice, causal attention, partial output
projection); the host sums the head-group/pair-couple partials per batch and
adds bproj. No device collectives.

x arrives host-pretransposed as xT[C, T] (fp16), so qkv matmuls stream it
directly with the contraction dim on partitions -- no on-device transposes.
Attention uses the transposed-scores layout sT[tk, tq]: softmax denominators
come out of the PV matmul via an extra ones column interleaved into Wv, and
are broadcast across partitions with a partition-step-0 SBUF->SBUF DMA.
Partial projection outputs are written fp16 and summed on the host.
"""

import numpy as np

import concourse.bass as bass
import concourse.tile as tile
from concourse import bacc, mybir
from concourse.bass_utils import run_bass_kernel_spmd

F32 = mybir.dt.float32
F32R = mybir.dt.float32r
F16 = mybir.dt.float16

N_CORES = 8
B, T, C = 4, 2048, 1024
NH_TOT, D = 16, 64
F = 512            # features per core (8 heads)
NH = 8             # local heads
NPAIR = 4          # head pairs (128 feats each)
CCH = C // 128     # 8 contraction chunks
NTT = T // 128     # 16 t tiles
NTB = T // 512     # 4 t blocks (qkv production)
NQB = T // 512     # 4 q blocks (attention)
VW = NH * (D + 1)  # 520: augmented v width
ADD = mybir.AluOpType.add
MULT = mybir.AluOpType.mult


def _emit(tc, aps):
    from contextlib import ExitStack
    nc = tc.nc
    x, wq, wk, wva, bq, bk, wp = (
        aps["x"], aps["wq"], aps["wk"], aps["wva"], aps["bq"], aps["bk"],
        aps["wp"])
    out_ab = [aps["out_pa"], aps["out_pb"]]

    # ---- pools (all coexist; ~210KB/partition total) ----
    ctx = ExitStack()
    pp_qk = ctx.enter_context(tc.tile_pool(name="ps_qk", bufs=2, space="PSUM"))
    pp_s = ctx.enter_context(tc.tile_pool(name="ps_s", bufs=2, space="PSUM"))
    pp_pv = ctx.enter_context(tc.tile_pool(name="ps_pv", bufs=1, space="PSUM"))
    po_v = ctx.enter_context(tc.tile_pool(name="v_all", bufs=1))
    po_mask = ctx.enter_context(tc.tile_pool(name="mask", bufs=1))
    po_wv = ctx.enter_context(tc.tile_pool(name="wv", bufs=16))
    po_qkt = ctx.enter_context(tc.tile_pool(name="qkT", bufs=2))
    po_bias = ctx.enter_context(tc.tile_pool(name="bias", bufs=1))
    po_misc = ctx.enter_context(tc.tile_pool(name="misc", bufs=3))
    po_xt = ctx.enter_context(tc.tile_pool(name="xT", bufs=1))
    po_wqk = ctx.enter_context(tc.tile_pool(name="wqk", bufs=2))
    po_yt = ctx.enter_context(tc.tile_pool(name="yT", bufs=4))
    po_exp = ctx.enter_context(tc.tile_pool(name="expT", bufs=4))
    po_rec = ctx.enter_context(tc.tile_pool(name="recip", bufs=3))
    po_den = ctx.enter_context(tc.tile_pool(name="den", bufs=2))
    po_ytmp = ctx.enter_context(tc.tile_pool(name="ytmp", bufs=2))
    po_wp = ctx.enter_context(tc.tile_pool(name="wp", bufs=4))

    # emask[p, w] = -6e4 if w < 384 + p else 0: sliced at [384-off, 512)
    # it masks the diagonal 128-col triangle plus the garbage strip, and
    # is ACCUMULATED onto scores by the PE itself (identity stationary)
    # so the softmax chain never hops through the vector engine
    emask_sb = po_mask.tile([128, 512], F16, tag="emask")
    nc.scalar.dma_start(out=emask_sb[:], in_=aps["emask"][:])
    ident = po_bias.tile([128, 128], F16, tag="ident")
    nc.scalar.dma_start(out=ident[:], in_=aps["identin"][:])
    ones_sb = po_bias.tile([128, 64], F16, tag="ones64")
    nc.scalar.dma_start(out=ones_sb[:], in_=aps["ones64"][:])
    # bva broadcast to all 128 partitions straight from DRAM
    bva_bc = po_bias.tile([128, VW], F32, tag="bva_bc")
    bva2 = aps["bva2"]
    nc.scalar.dma_start(out=bva_bc[:], in_=bass.AP(
        tensor=bva2.tensor, offset=bva2.offset,
        ap=[[0, 128]] + [list(a) for a in bva2.ap[1:]]))

    # ---- phase 0: pair-0 qkv weights go first on the sync ring; the
    # 4MB xT is split across BOTH HWDGE rings (even chunks sync, odd
    # chunks scalar) for 2x delivery rate, wv after
    xT = [po_xt.tile([128, T], F16, tag=f"xT{c}", name=f"xT{c}")
          for c in range(CCH)]
    wv_sb = [[None] * CCH, [None] * CCH]

    def load_xt():
        for c in range(CCH):
            eng = nc.sync if c % 2 == 0 else nc.scalar
            eng.dma_start(out=xT[c][:], in_=x[c * 128:(c + 1) * 128, :])

    def load_wv():
        for c in range(CCH):
            for half in range(2):
                cs = slice(half * 260, half * 260 + 260)
                wt = po_wv.tile([128, 260], F16, tag="wv")
                nc.scalar.dma_start(out=wt[:],
                                    in_=wva[c * 128:(c + 1) * 128, cs])
                wv_sb[half][c] = wt

    # ---- phase 0b: v (augmented with ones columns, all 8 heads) ----
    # half 0 = heads 0-3 (pairs 0,1), half 1 = heads 4-7 (pairs 2,3);
    # half 1 production overlaps pair-0 attention. tt pairs alternate
    # psum banks so consecutive matmuls never accumulate into the same
    # bank back-to-back
    v_all = [po_v.tile([128, VW], F16, tag=f"v{tt}", name=f"v{tt}")
             for tt in range(NTT)]

    def v_units(half, split=1):
        cs = slice(half * 260, half * 260 + 260)
        units = []
        for tt0 in range(0, NTT, 2):
            stt = {}

            def part(tt0=tt0, cs=cs, half=half, stt=stt, c0=0, c1=CCH,
                     fin=True):
                if c0 == 0:
                    stt["ps0"] = pp_qk.tile([128, 260], F32, tag="qk",
                                            name="ps0")
                    stt["ps1"] = pp_qk.tile([128, 260], F32, tag="qk",
                                            name="ps1")
                ps0, ps1 = stt["ps0"], stt["ps1"]
                for c in range(c0, c1):
                    nc.tensor.matmul(
                        ps0[:], xT[c][:, tt0 * 128:(tt0 + 1) * 128],
                        wv_sb[half][c][:], start=(c == 0),
                        stop=(c == CCH - 1))
                    nc.tensor.matmul(
                        ps1[:], xT[c][:, (tt0 + 1) * 128:(tt0 + 2) * 128],
                        wv_sb[half][c][:], start=(c == 0),
                        stop=(c == CCH - 1))
                if fin:
                    nc.vector.tensor_add(v_all[tt0][:, cs], ps0[:],
                                         bva_bc[:, cs])
                    nc.vector.tensor_add(v_all[tt0 + 1][:, cs], ps1[:],
                                         bva_bc[:, cs])

            if split == 1:
                units.append(part)
            else:
                from functools import partial
                units.append(partial(part, c0=0, c1=4, fin=False))
                units.append(partial(part, c0=4, c1=CCH, fin=True))
        return units

    # ---- per head pair: qkv -> attention -> partial proj ----
    # Emitted as interleaved work units so the PE instruction stream mixes
    # next-pair qkv (and couple proj) matmuls between attention groups --
    # engines are in-order, so a blocked exp-wait would otherwise stall
    # ready qkv work behind it.

    def prep_qkv(pair):
        psl = slice(pair * 128, (pair + 1) * 128)
        # one [128, 8*256] tile: chunk c at cols [256c, 256c+128) = wq,
        # [256c+128, 256(c+1)) = wk. Loaded with TWO strided DMAs (the
        # 16 little per-chunk DMAs serialize ~1us each on the ring)
        wqk_all = po_wqk.tile([128, 256 * CCH], F16, tag="wqk",
                              name="wqk_all")
        dap = wqk_all[:]
        dstep = dap.ap[1][0]
        for which, w in ((0, wq), (1, wk)):
            src = w[:, psl]
            sstep_r, sstep_e = src.ap[0][0], src.ap[1][0]
            nc.sync.dma_start(
                out=bass.AP(
                    tensor=dap.tensor,
                    offset=dap.offset + which * 128 * dstep,
                    ap=[list(dap.ap[0]),
                        [256 * dstep, CCH], [dstep, 128]]),
                in_=bass.AP(
                    tensor=src.tensor, offset=src.offset,
                    ap=[[sstep_r, 128], [sstep_r * 128, CCH],
                        [sstep_e, 128]]))
        wqk_c = [wqk_all[:, 256 * c:256 * (c + 1)] for c in range(CCH)]
        bq_sb = po_bias.tile([128, 1], F32, tag=f"bq{pair}", name=f"bq{pair}")
        nc.sync.dma_start(out=bq_sb[:], in_=bq[psl, :])
        bk_sb = po_bias.tile([128, 1], F32, tag=f"bk{pair}", name=f"bk{pair}")
        nc.sync.dma_start(out=bk_sb[:], in_=bk[psl, :])
        qT = po_qkt.tile([128, T], F16, tag="qT", name="qT")
        kT = po_qkt.tile([128, T], F16, tag="kT", name="kT")
        return dict(wqk=wqk_c, bq=bq_sb, bk=bk_sb, qT=qT, kT=kT)

    def qkv_units(st8, split=1):
        # split=2 yields two micro-closures per t-block (for weaving
        # between attention steps); psum tile lifetime spans the pair,
        # so micros of one t-block must stay adjacent in their stream
        units = []
        for tb in range(NTB):
            tsl = slice(tb * 512, (tb + 1) * 512)
            stt = {}

            def half(tb=tb, tsl=tsl, stt=stt, c0=0, c1=CCH, fin=True):
                if c0 == 0:
                    stt["psq"] = pp_qk.tile([128, 512], F32, tag="qk",
                                            name="psq")
                    stt["psk"] = pp_qk.tile([128, 512], F32, tag="qk",
                                            name="psk")
                psq, psk = stt["psq"], stt["psk"]
                # q/k matmuls interleaved so consecutive matmuls target
                # alternating psum banks
                for c in range(c0, c1):
                    nc.tensor.matmul(psq[:], st8["wqk"][c][:, 0:128],
                                     xT[c][:, tsl],
                                     start=(c == 0), stop=(c == CCH - 1))
                    nc.tensor.matmul(psk[:], st8["wqk"][c][:, 128:256],
                                     xT[c][:, tsl],
                                     start=(c == 0), stop=(c == CCH - 1))
                if fin:
                    # psum*1/sqrt(D) + bq/sqrt(D)  (bq pre-scaled on host)
                    nc.vector.tensor_scalar(
                        out=st8["qT"][:, tsl], in0=psq[:], scalar1=0.125,
                        scalar2=st8["bq"][:], op0=MULT, op1=ADD)
                    nc.vector.tensor_scalar(
                        out=st8["kT"][:, tsl], in0=psk[:],
                        scalar1=st8["bk"][:], scalar2=None, op0=ADD)

            if split == 1:
                units.append(half)
            else:
                from functools import partial
                units.append(partial(half, c0=0, c1=4, fin=False))
                units.append(partial(half, c0=4, c1=CCH, fin=True))
        return units

    def attn_micros(st8, yt):
        # One q-block unit computes BOTH heads of the pair: the two
        # score matmuls contract over disjoint 64-partition halves
        # (rows 0-63 = head hl0, rows 64-127 = head hl1) so the PE runs
        # them CONCURRENTLY in separate row groups. One st tile packs
        # [hl0 scores | hl1 scores] so a single wide exp covers both.
        # Returns kt-granular micro-closures (for weaving fillers into
        # the exp-latency slack) plus per-unit end indices for gating.
        qT, kT = st8["qT"], st8["kT"]
        micros = []
        ends = []
        for qb in range(NQB):
            nkt = 4 * qb + 4
            ustate = {}

            def emit_scores(kt, st, qb=qb):
                j = kt - 4 * qb
                off = 128 * j if j > 0 else 0
                diag = j >= 0
                ktw = slice(kt * 128, (kt + 1) * 128)
                qw = slice(qb * 512 + off, (qb + 1) * 512)
                nc.tensor.matmul(st[:, off:512], kT[0:64, ktw],
                                 qT[0:64, qw], start=True, stop=not diag)
                if diag:
                    # accumulate the causal mask (and, for hl1 below, the
                    # garbage strip) on the PE: identity stationary x
                    # constant emask moving
                    nc.tensor.matmul(st[:, off:off + 128], ident[:],
                                     emask_sb[:, 384:512],
                                     start=False, stop=True)
                nc.tensor.matmul(st[:, 512 + off:1024], kT[64:128, ktw],
                                 qT[64:128, qw], start=True, stop=not diag)
                if diag:
                    nc.tensor.matmul(st[:, 512:512 + off + 128], ident[:],
                                     emask_sb[:, 384 - off:512],
                                     start=False, stop=True)
                return off

            def emit_exp_pv(kt, st, off, qb=qb, nkt=nkt, ustate=ustate):
                pair = st8["pair"]
                vslA = slice((pair * 2) * 65, (pair * 2) * 65 + 65)
                vslB = slice((pair * 2 + 1) * 65, (pair * 2 + 1) * 65 + 65)
                et = po_exp.tile([128, 1024], F16, tag="expT", name="et")
                nc.scalar.activation(
                    et[:, off:1024], st[:, off:1024],
                    mybir.ActivationFunctionType.Exp)
                nc.tensor.matmul(ustate["pvA"][0:65, off:512],
                                 v_all[kt][:, vslA], et[:, off:512],
                                 start=(kt == 0), stop=(kt == nkt - 1))
                nc.tensor.matmul(ustate["pvB"][0:65, off:512],
                                 v_all[kt][:, vslB], et[:, 512 + off:1024],
                                 start=(kt == 0), stop=(kt == nkt - 1))

            def step(kt, qb=qb, nkt=nkt, ustate=ustate, es=emit_scores,
                     ep=emit_exp_pv):
                if kt == 0:
                    ustate["pvA"] = pp_pv.tile([128, 512], F32, tag="pvA",
                                               name="pvA")
                    ustate["pvB"] = pp_pv.tile([128, 512], F32, tag="pvB",
                                               name="pvB")
                if kt < nkt:
                    st = pp_s.tile([128, 1024], F32, tag="s", name="st")
                    off = es(kt, st, qb=qb)
                    pend = ustate.get("pend")
                    if pend is not None:
                        ep(*pend)
                    ustate["pend"] = (kt, st, off)
                else:
                    ep(*ustate["pend"])
                    ustate["pend"] = None

            def tail(hl, qb=qb, ustate=ustate):
                # den row 64 -> SBUF -> PE ones-broadcast into a borrowed
                # psum bank (partitions 0-63) -> recip -> mul. No DMA in
                # the chain that gates pv-bank reuse of the next q-block.
                qsl = slice(qb * 512, (qb + 1) * 512)
                pv = ustate["pvA"] if hl == 0 else ustate["pvB"]
                den = po_den.tile([128, 512], F16, tag="den", name="den")
                # scalar engine: PSUM->SBUF staging off the vector queue
                nc.scalar.copy(den[64:65, :], pv[64:65, :])
                recps = pp_qk.tile([128, 512], F32, tag="qk", name="recps")
                nc.tensor.matmul(recps[0:64, :], ones_sb[64:65, :],
                                 den[64:65, :], start=True, stop=True,
                                 tile_position=(64, 0))
                rec = po_rec.tile([128, 512], F32, tag="recip", name="rec")
                nc.vector.reciprocal_approx_fast(rec[0:64, :],
                                                 recps[0:64, :])
                if hl == 0:
                    nc.vector.tensor_mul(yt[0:64, qsl], pv[0:64, :],
                                         rec[0:64, :])
                else:
                    # engines can't cross partitions; bounce via DMA
                    ytmp = po_ytmp.tile([128, 512], F16, tag="ytmp",
                                        name="ytmp")
                    nc.vector.tensor_mul(ytmp[0:64, :], pv[0:64, :],
                                         rec[0:64, :])
                    nc.gpsimd.dma_start(out=yt[64:128, qsl],
                                        in_=ytmp[0:64, :])

            from functools import partial
            for kt in range(nkt):
                micros.append(partial(step, kt))
            def last(ustate=ustate, step=step, tail=tail, nkt=nkt):
                step(nkt)
                tail(0)
            micros.append(last)
            micros.append(partial(tail, 1))
            ends.append(len(micros) - 1)
        return micros, ends

    def prep_proj(couple):
        wp_sb = []
        for pq in range(2):
            for cb in range(2):
                prow = (couple * 2 + pq) * 128
                wt = po_wp.tile([128, 512], F16, tag="wp", name="wpt")
                nc.sync.dma_start(
                    out=wt[:],
                    in_=wp[prow:prow + 128, cb * 512:(cb + 1) * 512])
                wp_sb.append(wt)
        return wp_sb

    def proj_micros(couple, wp_sb, yts, tts):
        out_p = out_ab[couple]
        micros = []
        for tt in tts:
            def micro(tt=tt):
                ot = po_misc.tile([128, C], F16, tag="misc", name="ot")
                ps0 = pp_qk.tile([128, 512], F32, tag="qk", name="pp0")
                ps1 = pp_qk.tile([128, 512], F32, tag="qk", name="pp1")
                pss = (ps0, ps1)
                # pq outer: the y stationary is reused across the two
                # cb matmuls, which alternate psum banks
                for pq in range(2):
                    for cb in range(2):
                        nc.tensor.matmul(
                            pss[cb][:],
                            yts[pq][:, tt * 128:(tt + 1) * 128],
                            wp_sb[pq * 2 + cb][:],
                            start=(pq == 0), stop=(pq == 1))
                for cb in range(2):
                    nc.vector.tensor_copy(
                        ot[:, cb * 512:(cb + 1) * 512], pss[cb][:])
                nc.sync.dma_start(
                    out=out_p[tt * 128:(tt + 1) * 128, :], in_=ot[:])
            micros.append(micro)
        return micros

    def round_robin(*streams):
        streams = [list(s) for s in streams if s]
        while any(streams):
            for s in streams:
                if s:
                    s.pop(0)()

    def weave(primary, fillers, gates=None, boost=()):
        # Spread filler micro-closures evenly between primary ones;
        # gates[i] = index into primary that must already be emitted
        # before fillers[i] may run. boost = primary indices (attention
        # unit tails) after which extra fillers are popped to cover the
        # den-bounce chain that gates pv-bank reuse.
        nf, npr = len(fillers), len(primary)
        r = nf / npr if npr else 0.0
        acc, fi = 0.0, 0
        boost = set(boost)
        for pi, u in enumerate(primary):
            u()
            acc += r
            if pi in boost:
                acc += 2.0
            while (fi < nf and acc >= 1.0
                   and (gates is None or gates[fi] <= pi)):
                fillers[fi]()
                fi += 1
                acc -= 1.0
        while fi < nf:
            fillers[fi]()
            fi += 1

    pair_state = []
    yts = []
    st0 = prep_qkv(0)
    st0["pair"] = 0
    pair_state.append(st0)
    load_xt()
    load_wv()
    # pair-0 qkv races ahead of v-half0 so pair-0 attention (which only
    # needs half-0 v) starts as early as possible; v-half1 (for pairs
    # 2,3) overlaps pair-0 attention
    round_robin(qkv_units(st0), v_units(0))
    # p0: attn0 + (qkv1, v-half1) fillers
    yt0 = po_yt.tile([128, T], F16, tag="yT", name="yt0")
    yts.append(yt0)
    am0, ends0 = attn_micros(pair_state[0], yt0)
    st1 = prep_qkv(1)
    st1["pair"] = 1
    pair_state.append(st1)
    f0 = qkv_units(st1) + v_units(1)
    weave(am0, f0, [-1] * len(f0), boost=ends0)

    # p1: attn1 + (qkv2, first half of couple-0 proj) fillers
    yt1 = po_yt.tile([128, T], F16, tag="yT", name="yt1")
    yts.append(yt1)
    am1, ends1 = attn_micros(pair_state[1], yt1)
    st2 = prep_qkv(2)
    st2["pair"] = 2
    pair_state.append(st2)
    wp_sb0 = prep_proj(0)
    f1 = qkv_units(st2)
    g1 = [-1] * len(f1)
    f1 += proj_micros(0, wp_sb0, yts[0:2], range(0, 8))
    g1 += [ends1[tt // 4] for tt in range(0, 8)]
    weave(am1, f1, g1, boost=ends1)

    # p2+p3 merged: one weave so fillers flow across the boundary
    yt2 = po_yt.tile([128, T], F16, tag="yT", name="yt2")
    yt3 = po_yt.tile([128, T], F16, tag="yT", name="yt3")
    yts += [yt2, yt3]
    st3 = prep_qkv(3)
    st3["pair"] = 3
    pair_state.append(st3)
    am2, ends2 = attn_micros(pair_state[2], yt2)
    am3, ends3 = attn_micros(pair_state[3], yt3)
    wp_sb1 = prep_proj(1)
    am23 = am2 + am3
    ends23 = list(ends2) + [len(am2) + e for e in ends3]
    f2 = qkv_units(st3)
    g2 = [-1] * len(f2)
    f2 += proj_micros(0, wp_sb0, yts[0:2], range(8, 16))
    g2 += [-1] * 8
    f2 += proj_micros(1, wp_sb1, yts[2:4], range(0, 16))
    g2 += [len(am2) + ends3[tt // 4] for tt in range(0, 16)]
    weave(am23, f2, g2, boost=ends23)

    ctx.close()


_CACHE = {}


def _build():
    if "nc" in _CACHE:
        return _CACHE["nc"]
    nc = bacc.Bacc("TRN2", target_bir_lowering=False, debug=False,
                   enable_asserts=True, num_devices=N_CORES)
    aps = {
        "x": nc.dram_tensor("x", [C, T], F16, kind="ExternalInput").ap(),
        "wq": nc.dram_tensor("wq", [C, F], F16, kind="ExternalInput").ap(),
        "wk": nc.dram_tensor("wk", [C, F], F16, kind="ExternalInput").ap(),
        "wva": nc.dram_tensor("wva", [C, VW], F16, kind="ExternalInput").ap(),
        "bq": nc.dram_tensor("bq", [F, 1], F32, kind="ExternalInput").ap(),
        "bk": nc.dram_tensor("bk", [F, 1], F32, kind="ExternalInput").ap(),
        "bva2": nc.dram_tensor("bva2", [1, VW], F32, kind="ExternalInput").ap(),
        "wp": nc.dram_tensor("wp", [F, C], F16, kind="ExternalInput").ap(),
        "emask": nc.dram_tensor("emask", [128, 512], F16,
                                kind="ExternalInput").ap(),
        "identin": nc.dram_tensor("identin", [128, 128], F16,
                                  kind="ExternalInput").ap(),
        "ones64": nc.dram_tensor("ones64", [128, 64], F16,
                                 kind="ExternalInput").ap(),
        "out_pa": nc.dram_tensor("out_pa", [T, C], F16,
                                 kind="ExternalOutput").ap(),
        "out_pb": nc.dram_tensor("out_pb", [T, C], F16,
                                 kind="ExternalOutput").ap(),
    }
    with tile.TileContext(nc) as tc:
        _emit(tc, aps)
    nc.compile()
    _CACHE["nc"] = nc
    return nc


def _make_in_maps(x, Wqkv, bqkv, Wproj):
    x = np.asarray(x, dtype=np.float32)
    Wqkv = np.asarray(Wqkv, dtype=np.float32)
    bqkv = np.asarray(bqkv, dtype=np.float32)
    Wproj = np.asarray(Wproj, dtype=np.float32)

    # emask[p, w] = -6e4 if w < 384 + p else 0 (accumulated onto scores
    # by the PE; -6e4 stays within f16 range and exp() underflows to 0)
    p_idx = np.arange(128)[:, None]
    w_idx = np.arange(512)[None, :]
    emask = np.where(w_idx < 384 + p_idx, -6e4, 0.0).astype(np.float16)

    in_maps = []
    for core in range(N_CORES):
        b, g = divmod(core, 2)
        q0, k0, v0 = 512 * g, C + 512 * g, 2 * C + 512 * g
        wva = np.zeros((C, VW), dtype=np.float32)
        bva = np.zeros((1, VW), dtype=np.float32)
        for h in range(NH):
            src = v0 + D * h
            dst = 65 * h
            # per-head layout [v(64), one]
            wva[:, dst:dst + 64] = Wqkv[:, src:src + 64]
            bva[0, dst:dst + 64] = bqkv[src:src + 64]
            bva[0, dst + 64] = 1.0
        in_maps.append({
            "x": np.ascontiguousarray(x[b].T).astype(np.float16),
            "wq": np.ascontiguousarray(Wqkv[:, q0:q0 + F]).astype(np.float16),
            "wk": np.ascontiguousarray(Wqkv[:, k0:k0 + F]).astype(np.float16),
            "wva": wva.astype(np.float16),
            "bq": np.ascontiguousarray(bqkv[q0:q0 + F].reshape(F, 1) * 0.125),
            "bk": np.ascontiguousarray(bqkv[k0:k0 + F].reshape(F, 1)),
            "bva2": bva,
            "wp": np.ascontiguousarray(Wproj[512 * g:512 * g + F, :]).astype(np.float16),
            "emask": emask,
            "identin": np.eye(128, dtype=np.float16),
            "ones64": np.ones((128, 64), dtype=np.float16),
        })
    return in_maps


def run_sharded(x, Wqkv, bqkv, Wproj, bproj, trace=False):
    nc = _build()
    in_maps = _make_in_maps(x, Wqkv, bqkv, Wproj)
    res = run_bass_kernel_spmd(nc, in_maps, core_ids=list(range(N_CORES)),
                               trace=trace)
    bproj = np.asarray(bproj, dtype=np.float32)
    out = np.empty((B, T, C), dtype=np.float32)
    for b in range(B):
        acc = bproj[None, :].astype(np.float32).repeat(T, axis=0)
        for core in (2 * b, 2 * b + 1):
            acc = (acc + res.results[core]["out_pa"].astype(np.float32)
                   + res.results[core]["out_pb"].astype(np.float32))
        out[b] = acc
    return out, res


def kernel(x, Wqkv, bqkv, Wproj, bproj):
    out, _ = run_sharded(x, Wqkv, bqkv, Wproj, bproj, trace=False)
    return out



# revision 53
# speedup vs baseline: 1.0792x; 1.0254x over previous
"""Causal self-attention (B=4, T=2048, C=1024, H=16, Dh=64) on 8 trn2 NeuronCores.

Sharding: core i <-> (batch b = i//2, head-group g = i%2). Each core computes
8 heads of one batch end-to-end (qkv slice, causal attention, partial output
projection); the host sums the head-group/pair-couple partials per batch and
adds bproj. No device collectives.

x arrives host-pretransposed as xT[C, T] (fp16), so qkv matmuls stream it
directly with the contraction dim on partitions -- no on-device transposes.
Attention uses the transposed-scores layout sT[tk, tq]: softmax denominators
come out of the PV matmul via an extra ones column interleaved into Wv, and
are broadcast across partitions with a partition-step-0 SBUF->SBUF DMA.
Partial projection outputs are written fp16 and summed on the host.
"""

import numpy as np

import concourse.bass as bass
import concourse.tile as tile
from concourse import bacc, mybir
from concourse.bass_utils import run_bass_kernel_spmd

F32 = mybir.dt.float32
F32R = mybir.dt.float32r
F16 = mybir.dt.float16

N_CORES = 8
B, T, C = 4, 2048, 1024
NH_TOT, D = 16, 64
F = 512            # features per core (8 heads)
NH = 8             # local heads
NPAIR = 4          # head pairs (128 feats each)
CCH = C // 128     # 8 contraction chunks
NTT = T // 128     # 16 t tiles
NTB = T // 512     # 4 t blocks (qkv production)
NQB = T // 512     # 4 q blocks (attention)
VW = NH * (D + 1)  # 520: augmented v width
ADD = mybir.AluOpType.add
MULT = mybir.AluOpType.mult


def _emit(tc, aps):
    from contextlib import ExitStack
    nc = tc.nc
    x, wq, wk, wva, bq, bk, wp = (
        aps["x"], aps["wq"], aps["wk"], aps["wva"], aps["bq"], aps["bk"],
        aps["wp"])
    out_ab = [aps["out_pa"], aps["out_pb"]]

    # ---- pools (all coexist; ~210KB/partition total) ----
    ctx = ExitStack()
    pp_qk = ctx.enter_context(tc.tile_pool(name="ps_qk", bufs=2, space="PSUM"))
    pp_s = ctx.enter_context(tc.tile_pool(name="ps_s", bufs=2, space="PSUM"))
    pp_pv = ctx.enter_context(tc.tile_pool(name="ps_pv", bufs=1, space="PSUM"))
    po_v = ctx.enter_context(tc.tile_pool(name="v_all", bufs=1))
    po_mask = ctx.enter_context(tc.tile_pool(name="mask", bufs=1))
    po_wv = ctx.enter_context(tc.tile_pool(name="wv", bufs=16))
    po_qkt = ctx.enter_context(tc.tile_pool(name="qkT", bufs=2))
    po_bias = ctx.enter_context(tc.tile_pool(name="bias", bufs=1))
    po_misc = ctx.enter_context(tc.tile_pool(name="misc", bufs=3))
    po_xt = ctx.enter_context(tc.tile_pool(name="xT", bufs=1))
    po_wqk = ctx.enter_context(tc.tile_pool(name="wqk", bufs=2))
    po_yt = ctx.enter_context(tc.tile_pool(name="yT", bufs=4))
    po_exp = ctx.enter_context(tc.tile_pool(name="expT", bufs=4))
    po_rec = ctx.enter_context(tc.tile_pool(name="recip", bufs=3))
    po_den = ctx.enter_context(tc.tile_pool(name="den", bufs=2))
    po_ytmp = ctx.enter_context(tc.tile_pool(name="ytmp", bufs=2))
    po_wp = ctx.enter_context(tc.tile_pool(name="wp", bufs=4))

    # emask[p, w] = -6e4 if w < 384 + p else 0: sliced at [384-off, 512)
    # it masks the diagonal 128-col triangle plus the garbage strip, and
    # is ACCUMULATED onto scores by the PE itself (identity stationary)
    # so the softmax chain never hops through the vector engine
    emask_sb = po_mask.tile([128, 512], F16, tag="emask")
    nc.scalar.dma_start(out=emask_sb[:], in_=aps["emask"][:])
    ident = po_bias.tile([128, 128], F16, tag="ident")
    nc.scalar.dma_start(out=ident[:], in_=aps["identin"][:])
    ones_sb = po_bias.tile([128, 64], F16, tag="ones64")
    nc.scalar.dma_start(out=ones_sb[:], in_=aps["ones64"][:])
    # bva broadcast to all 128 partitions straight from DRAM
    bva_bc = po_bias.tile([128, VW], F32, tag="bva_bc")
    bva2 = aps["bva2"]
    nc.scalar.dma_start(out=bva_bc[:], in_=bass.AP(
        tensor=bva2.tensor, offset=bva2.offset,
        ap=[[0, 128]] + [list(a) for a in bva2.ap[1:]]))

    # ---- phase 0: pair-0 qkv weights go first on the sync ring; the
    # 4MB xT is split across BOTH HWDGE rings (even chunks sync, odd
    # chunks scalar) for 2x delivery rate, wv after
    xT = [po_xt.tile([128, T], F16, tag=f"xT{c}", name=f"xT{c}")
          for c in range(CCH)]
    wv_sb = [[None] * CCH, [None] * CCH]

    def load_xt():
        # t-block-major so pair-0 qkv (t block 0) unblocks after ~1MB
        # instead of the full 4MB
        for tb in range(NTB):
            tsl = slice(tb * 512, (tb + 1) * 512)
            for c in range(CCH):
                eng = nc.sync if c % 2 == 0 else nc.scalar
                eng.dma_start(out=xT[c][:, tsl],
                              in_=x[c * 128:(c + 1) * 128, tsl])

    def load_wv():
        for c in range(CCH):
            for half in range(2):
                cs = slice(half * 260, half * 260 + 260)
                wt = po_wv.tile([128, 260], F16, tag="wv")
                nc.scalar.dma_start(out=wt[:],
                                    in_=wva[c * 128:(c + 1) * 128, cs])
                wv_sb[half][c] = wt

    # ---- phase 0b: v (augmented with ones columns, all 8 heads) ----
    # half 0 = heads 0-3 (pairs 0,1), half 1 = heads 4-7 (pairs 2,3);
    # half 1 production overlaps pair-0 attention. tt pairs alternate
    # psum banks so consecutive matmuls never accumulate into the same
    # bank back-to-back
    v_all = [po_v.tile([128, VW], F16, tag=f"v{tt}", name=f"v{tt}")
             for tt in range(NTT)]

    def v_units(half, split=1):
        cs = slice(half * 260, half * 260 + 260)
        units = []
        for tt0 in range(0, NTT, 2):
            stt = {}

            def part(tt0=tt0, cs=cs, half=half, stt=stt, c0=0, c1=CCH,
                     fin=True):
                if c0 == 0:
                    stt["ps0"] = pp_qk.tile([128, 260], F32, tag="qk",
                                            name="ps0")
                    stt["ps1"] = pp_qk.tile([128, 260], F32, tag="qk",
                                            name="ps1")
                ps0, ps1 = stt["ps0"], stt["ps1"]
                for c in range(c0, c1):
                    nc.tensor.matmul(
                        ps0[:], xT[c][:, tt0 * 128:(tt0 + 1) * 128],
                        wv_sb[half][c][:], start=(c == 0),
                        stop=(c == CCH - 1))
                    nc.tensor.matmul(
                        ps1[:], xT[c][:, (tt0 + 1) * 128:(tt0 + 2) * 128],
                        wv_sb[half][c][:], start=(c == 0),
                        stop=(c == CCH - 1))
                if fin:
                    nc.vector.tensor_add(v_all[tt0][:, cs], ps0[:],
                                         bva_bc[:, cs])
                    nc.vector.tensor_add(v_all[tt0 + 1][:, cs], ps1[:],
                                         bva_bc[:, cs])

            if split == 1:
                units.append(part)
            else:
                from functools import partial
                units.append(partial(part, c0=0, c1=4, fin=False))
                units.append(partial(part, c0=4, c1=CCH, fin=True))
        return units

    # ---- per head pair: qkv -> attention -> partial proj ----
    # Emitted as interleaved work units so the PE instruction stream mixes
    # next-pair qkv (and couple proj) matmuls between attention groups --
    # engines are in-order, so a blocked exp-wait would otherwise stall
    # ready qkv work behind it.

    def prep_qkv(pair):
        psl = slice(pair * 128, (pair + 1) * 128)
        # one [128, 8*256] tile: chunk c at cols [256c, 256c+128) = wq,
        # [256c+128, 256(c+1)) = wk. Loaded with TWO strided DMAs (the
        # 16 little per-chunk DMAs serialize ~1us each on the ring)
        wqk_all = po_wqk.tile([128, 256 * CCH], F16, tag="wqk",
                              name="wqk_all")
        dap = wqk_all[:]
        dstep = dap.ap[1][0]
        for which, w in ((0, wq), (1, wk)):
            src = w[:, psl]
            sstep_r, sstep_e = src.ap[0][0], src.ap[1][0]
            nc.sync.dma_start(
                out=bass.AP(
                    tensor=dap.tensor,
                    offset=dap.offset + which * 128 * dstep,
                    ap=[list(dap.ap[0]),
                        [256 * dstep, CCH], [dstep, 128]]),
                in_=bass.AP(
                    tensor=src.tensor, offset=src.offset,
                    ap=[[sstep_r, 128], [sstep_r * 128, CCH],
                        [sstep_e, 128]]))
        wqk_c = [wqk_all[:, 256 * c:256 * (c + 1)] for c in range(CCH)]
        bq_sb = po_bias.tile([128, 1], F32, tag=f"bq{pair}", name=f"bq{pair}")
        nc.sync.dma_start(out=bq_sb[:], in_=bq[psl, :])
        bk_sb = po_bias.tile([128, 1], F32, tag=f"bk{pair}", name=f"bk{pair}")
        nc.sync.dma_start(out=bk_sb[:], in_=bk[psl, :])
        qT = po_qkt.tile([128, T], F16, tag="qT", name="qT")
        kT = po_qkt.tile([128, T], F16, tag="kT", name="kT")
        return dict(wqk=wqk_c, bq=bq_sb, bk=bk_sb, qT=qT, kT=kT)

    def qkv_units(st8, split=1):
        # split=2 yields two micro-closures per t-block (for weaving
        # between attention steps); psum tile lifetime spans the pair,
        # so micros of one t-block must stay adjacent in their stream
        units = []
        for tb in range(NTB):
            tsl = slice(tb * 512, (tb + 1) * 512)
            stt = {}

            def half(tb=tb, tsl=tsl, stt=stt, c0=0, c1=CCH, fin=True):
                if c0 == 0:
                    stt["psq"] = pp_qk.tile([128, 512], F32, tag="qk",
                                            name="psq")
                    stt["psk"] = pp_qk.tile([128, 512], F32, tag="qk",
                                            name="psk")
                psq, psk = stt["psq"], stt["psk"]
                # q/k matmuls interleaved so consecutive matmuls target
                # alternating psum banks
                for c in range(c0, c1):
                    nc.tensor.matmul(psq[:], st8["wqk"][c][:, 0:128],
                                     xT[c][:, tsl],
                                     start=(c == 0), stop=(c == CCH - 1))
                    nc.tensor.matmul(psk[:], st8["wqk"][c][:, 128:256],
                                     xT[c][:, tsl],
                                     start=(c == 0), stop=(c == CCH - 1))
                if fin:
                    # psum*1/sqrt(D) + bq/sqrt(D)  (bq pre-scaled on host)
                    nc.vector.tensor_scalar(
                        out=st8["qT"][:, tsl], in0=psq[:], scalar1=0.125,
                        scalar2=st8["bq"][:], op0=MULT, op1=ADD)
                    nc.vector.tensor_scalar(
                        out=st8["kT"][:, tsl], in0=psk[:],
                        scalar1=st8["bk"][:], scalar2=None, op0=ADD)

            if split == 1:
                units.append(half)
            else:
                from functools import partial
                units.append(partial(half, c0=0, c1=4, fin=False))
                units.append(partial(half, c0=4, c1=CCH, fin=True))
        return units

    def attn_micros(st8, yt):
        # One q-block unit computes BOTH heads of the pair: the two
        # score matmuls contract over disjoint 64-partition halves
        # (rows 0-63 = head hl0, rows 64-127 = head hl1) so the PE runs
        # them CONCURRENTLY in separate row groups. One st tile packs
        # [hl0 scores | hl1 scores] so a single wide exp covers both.
        # Returns kt-granular micro-closures (for weaving fillers into
        # the exp-latency slack) plus per-unit end indices for gating.
        qT, kT = st8["qT"], st8["kT"]
        micros = []
        ends = []
        for qb in range(NQB):
            nkt = 4 * qb + 4
            ustate = {}

            def emit_scores(kt, st, qb=qb):
                j = kt - 4 * qb
                off = 128 * j if j > 0 else 0
                diag = j >= 0
                ktw = slice(kt * 128, (kt + 1) * 128)
                qw = slice(qb * 512 + off, (qb + 1) * 512)
                nc.tensor.matmul(st[:, off:512], kT[0:64, ktw],
                                 qT[0:64, qw], start=True, stop=not diag)
                if diag:
                    # accumulate the causal mask (and, for hl1 below, the
                    # garbage strip) on the PE: identity stationary x
                    # constant emask moving
                    nc.tensor.matmul(st[:, off:off + 128], ident[:],
                                     emask_sb[:, 384:512],
                                     start=False, stop=True)
                nc.tensor.matmul(st[:, 512 + off:1024], kT[64:128, ktw],
                                 qT[64:128, qw], start=True, stop=not diag)
                if diag:
                    nc.tensor.matmul(st[:, 512:512 + off + 128], ident[:],
                                     emask_sb[:, 384 - off:512],
                                     start=False, stop=True)
                return off

            def emit_exp_pv(kt, st, off, qb=qb, nkt=nkt, ustate=ustate):
                pair = st8["pair"]
                vslA = slice((pair * 2) * 65, (pair * 2) * 65 + 65)
                vslB = slice((pair * 2 + 1) * 65, (pair * 2 + 1) * 65 + 65)
                et = po_exp.tile([128, 1024], F16, tag="expT", name="et")
                nc.scalar.activation(
                    et[:, off:1024], st[:, off:1024],
                    mybir.ActivationFunctionType.Exp)
                nc.tensor.matmul(ustate["pvA"][0:65, off:512],
                                 v_all[kt][:, vslA], et[:, off:512],
                                 start=(kt == 0), stop=(kt == nkt - 1))
                nc.tensor.matmul(ustate["pvB"][0:65, off:512],
                                 v_all[kt][:, vslB], et[:, 512 + off:1024],
                                 start=(kt == 0), stop=(kt == nkt - 1))

            def step(kt, qb=qb, nkt=nkt, ustate=ustate, es=emit_scores,
                     ep=emit_exp_pv):
                if kt == 0:
                    ustate["pvA"] = pp_pv.tile([128, 512], F32, tag="pvA",
                                               name="pvA")
                    ustate["pvB"] = pp_pv.tile([128, 512], F32, tag="pvB",
                                               name="pvB")
                if kt < nkt:
                    st = pp_s.tile([128, 1024], F32, tag="s", name="st")
                    off = es(kt, st, qb=qb)
                    pend = ustate.get("pend")
                    if pend is not None:
                        ep(*pend)
                    ustate["pend"] = (kt, st, off)
                else:
                    ep(*ustate["pend"])
                    ustate["pend"] = None

            def tail(hl, qb=qb, ustate=ustate):
                # den row 64 -> SBUF -> PE ones-broadcast into a borrowed
                # psum bank (partitions 0-63) -> recip -> mul. No DMA in
                # the chain that gates pv-bank reuse of the next q-block.
                qsl = slice(qb * 512, (qb + 1) * 512)
                pv = ustate["pvA"] if hl == 0 else ustate["pvB"]
                den = po_den.tile([128, 512], F16, tag="den", name="den")
                nc.vector.tensor_copy(den[64:65, :], pv[64:65, :])
                recps = pp_qk.tile([128, 512], F32, tag="qk", name="recps")
                nc.tensor.matmul(recps[0:64, :], ones_sb[64:65, :],
                                 den[64:65, :], start=True, stop=True,
                                 tile_position=(64, 0))
                rec = po_rec.tile([128, 512], F32, tag="recip", name="rec")
                nc.vector.reciprocal_approx_fast(rec[0:64, :],
                                                 recps[0:64, :])
                if hl == 0:
                    nc.vector.tensor_mul(yt[0:64, qsl], pv[0:64, :],
                                         rec[0:64, :])
                else:
                    # engines can't cross partitions; bounce via DMA
                    ytmp = po_ytmp.tile([128, 512], F16, tag="ytmp",
                                        name="ytmp")
                    nc.vector.tensor_mul(ytmp[0:64, :], pv[0:64, :],
                                         rec[0:64, :])
                    nc.gpsimd.dma_start(out=yt[64:128, qsl],
                                        in_=ytmp[0:64, :])

            from functools import partial
            for kt in range(nkt):
                micros.append(partial(step, kt))
            def last(ustate=ustate, step=step, tail=tail, nkt=nkt):
                step(nkt)
                tail(0)
            micros.append(last)
            micros.append(partial(tail, 1))
            ends.append(len(micros) - 1)
        return micros, ends

    def prep_proj(couple):
        wp_sb = []
        for pq in range(2):
            for cb in range(2):
                prow = (couple * 2 + pq) * 128
                wt = po_wp.tile([128, 512], F16, tag="wp", name="wpt")
                nc.sync.dma_start(
                    out=wt[:],
                    in_=wp[prow:prow + 128, cb * 512:(cb + 1) * 512])
                wp_sb.append(wt)
        return wp_sb

    def proj_micros(couple, wp_sb, yts, tts):
        out_p = out_ab[couple]
        micros = []
        for tt in tts:
            def micro(tt=tt):
                ot = po_misc.tile([128, C], F16, tag="misc", name="ot")
                ps0 = pp_qk.tile([128, 512], F32, tag="qk", name="pp0")
                ps1 = pp_qk.tile([128, 512], F32, tag="qk", name="pp1")
                pss = (ps0, ps1)
                # pq outer: the y stationary is reused across the two
                # cb matmuls, which alternate psum banks
                for pq in range(2):
                    for cb in range(2):
                        nc.tensor.matmul(
                            pss[cb][:],
                            yts[pq][:, tt * 128:(tt + 1) * 128],
                            wp_sb[pq * 2 + cb][:],
                            start=(pq == 0), stop=(pq == 1))
                for cb in range(2):
                    nc.vector.tensor_copy(
                        ot[:, cb * 512:(cb + 1) * 512], pss[cb][:])
                nc.sync.dma_start(
                    out=out_p[tt * 128:(tt + 1) * 128, :], in_=ot[:])
            micros.append(micro)
        return micros

    def round_robin(*streams):
        streams = [list(s) for s in streams if s]
        while any(streams):
            for s in streams:
                if s:
                    s.pop(0)()

    def weave(primary, fillers, gates=None, boost=()):
        # Spread filler micro-closures evenly between primary ones;
        # gates[i] = index into primary that must already be emitted
        # before fillers[i] may run. boost = primary indices (attention
        # unit tails) after which extra fillers are popped to cover the
        # den-bounce chain that gates pv-bank reuse.
        nf, npr = len(fillers), len(primary)
        r = nf / npr if npr else 0.0
        acc, fi = 0.0, 0
        boost = set(boost)
        for pi, u in enumerate(primary):
            u()
            acc += r
            if pi in boost:
                acc += 2.0
            while (fi < nf and acc >= 1.0
                   and (gates is None or gates[fi] <= pi)):
                fillers[fi]()
                fi += 1
                acc -= 1.0
        while fi < nf:
            fillers[fi]()
            fi += 1

    pair_state = []
    yts = []
    st0 = prep_qkv(0)
    st0["pair"] = 0
    pair_state.append(st0)
    load_xt()
    load_wv()
    # pair-0 qkv races ahead of v-half0 so pair-0 attention (which only
    # needs half-0 v) starts as early as possible; v-half1 (for pairs
    # 2,3) overlaps pair-0 attention
    round_robin(qkv_units(st0), v_units(0))
    # p0: attn0 + (qkv1, v-half1) fillers
    yt0 = po_yt.tile([128, T], F16, tag="yT", name="yt0")
    yts.append(yt0)
    am0, ends0 = attn_micros(pair_state[0], yt0)
    st1 = prep_qkv(1)
    st1["pair"] = 1
    pair_state.append(st1)
    f0 = qkv_units(st1) + v_units(1)
    weave(am0, f0, [-1] * len(f0), boost=ends0)

    # p1: attn1 + (qkv2, first half of couple-0 proj) fillers
    yt1 = po_yt.tile([128, T], F16, tag="yT", name="yt1")
    yts.append(yt1)
    am1, ends1 = attn_micros(pair_state[1], yt1)
    st2 = prep_qkv(2)
    st2["pair"] = 2
    pair_state.append(st2)
    wp_sb0 = prep_proj(0)
    f1 = qkv_units(st2)
    g1 = [-1] * len(f1)
    f1 += proj_micros(0, wp_sb0, yts[0:2], range(0, 8))
    g1 += [ends1[tt // 4] for tt in range(0, 8)]
    weave(am1, f1, g1, boost=ends1)

    # p2+p3 merged: one weave so fillers flow across the boundary
    yt2 = po_yt.tile([128, T], F16, tag="yT", name="yt2")
    yt3 = po_yt.tile([128, T], F16, tag="yT", name="yt3")
    yts += [yt2, yt3]
    st3 = prep_qkv(3)
    st3["pair"] = 3
    pair_state.append(st3)
    am2, ends2 = attn_micros(pair_state[2], yt2)
    am3, ends3 = attn_micros(pair_state[3], yt3)
    wp_sb1 = prep_proj(1)
    am23 = am2 + am3
    ends23 = list(ends2) + [len(am2) + e for e in ends3]
    f2 = qkv_units(st3)
    g2 = [-1] * len(f2)
    f2 += proj_micros(0, wp_sb0, yts[0:2], range(8, 16))
    g2 += [-1] * 8
    f2 += proj_micros(1, wp_sb1, yts[2:4], range(0, 16))
    g2 += [len(am2) + ends3[tt // 4] for tt in range(0, 16)]
    weave(am23, f2, g2, boost=ends23)

    ctx.close()


_CACHE = {}


def _build():
    if "nc" in _CACHE:
        return _CACHE["nc"]
    nc = bacc.Bacc("TRN2", target_bir_lowering=False, debug=False,
                   enable_asserts=True, num_devices=N_CORES)
    aps = {
        "x": nc.dram_tensor("x", [C, T], F16, kind="ExternalInput").ap(),
        "wq": nc.dram_tensor("wq", [C, F], F16, kind="ExternalInput").ap(),
        "wk": nc.dram_tensor("wk", [C, F], F16, kind="ExternalInput").ap(),
        "wva": nc.dram_tensor("wva", [C, VW], F16, kind="ExternalInput").ap(),
        "bq": nc.dram_tensor("bq", [F, 1], F32, kind="ExternalInput").ap(),
        "bk": nc.dram_tensor("bk", [F, 1], F32, kind="ExternalInput").ap(),
        "bva2": nc.dram_tensor("bva2", [1, VW], F32, kind="ExternalInput").ap(),
        "wp": nc.dram_tensor("wp", [F, C], F16, kind="ExternalInput").ap(),
        "emask": nc.dram_tensor("emask", [128, 512], F16,
                                kind="ExternalInput").ap(),
        "identin": nc.dram_tensor("identin", [128, 128], F16,
                                  kind="ExternalInput").ap(),
        "ones64": nc.dram_tensor("ones64", [128, 64], F16,
                                 kind="ExternalInput").ap(),
        "out_pa": nc.dram_tensor("out_pa", [T, C], F16,
                                 kind="ExternalOutput").ap(),
        "out_pb": nc.dram_tensor("out_pb", [T, C], F16,
                                 kind="ExternalOutput").ap(),
    }
    with tile.TileContext(nc) as tc:
        _emit(tc, aps)
    nc.compile()
    _CACHE["nc"] = nc
    return nc


def _make_in_maps(x, Wqkv, bqkv, Wproj):
    x = np.asarray(x, dtype=np.float32)
    Wqkv = np.asarray(Wqkv, dtype=np.float32)
    bqkv = np.asarray(bqkv, dtype=np.float32)
    Wproj = np.asarray(Wproj, dtype=np.float32)

    # emask[p, w] = -6e4 if w < 384 + p else 0 (accumulated onto scores
    # by the PE; -6e4 stays within f16 range and exp() underflows to 0)
    p_idx = np.arange(128)[:, None]
    w_idx = np.arange(512)[None, :]
    emask = np.where(w_idx < 384 + p_idx, -6e4, 0.0).astype(np.float16)

    in_maps = []
    for core in range(N_CORES):
        b, g = divmod(core, 2)
        q0, k0, v0 = 512 * g, C + 512 * g, 2 * C + 512 * g
        wva = np.zeros((C, VW), dtype=np.float32)
        bva = np.zeros((1, VW), dtype=np.float32)
        for h in range(NH):
            src = v0 + D * h
            dst = 65 * h
            # per-head layout [v(64), one]
            wva[:, dst:dst + 64] = Wqkv[:, src:src + 64]
            bva[0, dst:dst + 64] = bqkv[src:src + 64]
            bva[0, dst + 64] = 1.0
        in_maps.append({
            "x": np.ascontiguousarray(x[b].T).astype(np.float16),
            "wq": np.ascontiguousarray(Wqkv[:, q0:q0 + F]).astype(np.float16),
            "wk": np.ascontiguousarray(Wqkv[:, k0:k0 + F]).astype(np.float16),
            "wva": wva.astype(np.float16),
            "bq": np.ascontiguousarray(bqkv[q0:q0 + F].reshape(F, 1) * 0.125),
            "bk": np.ascontiguousarray(bqkv[k0:k0 + F].reshape(F, 1)),
            "bva2": bva,
            "wp": np.ascontiguousarray(Wproj[512 * g:512 * g + F, :]).astype(np.float16),
            "emask": emask,
            "identin": np.eye(128, dtype=np.float16),
            "ones64": np.ones((128, 64), dtype=np.float16),
        })
    return in_maps


def run_sharded(x, Wqkv, bqkv, Wproj, bproj, trace=False):
    nc = _build()
    in_maps = _make_in_maps(x, Wqkv, bqkv, Wproj)
    res = run_bass_kernel_spmd(nc, in_maps, core_ids=list(range(N_CORES)),
                               trace=trace)
    bproj = np.asarray(bproj, dtype=np.float32)
    out = np.empty((B, T, C), dtype=np.float32)
    for b in range(B):
        acc = bproj[None, :].astype(np.float32).repeat(T, axis=0)
        for core in (2 * b, 2 * b + 1):
            acc = (acc + res.results[core]["out_pa"].astype(np.float32)
                   + res.results[core]["out_pb"].astype(np.float32))
        out[b] = acc
    return out, res


def kernel(x, Wqkv, bqkv, Wproj, bproj):
    out, _ = run_sharded(x, Wqkv, bqkv, Wproj, bproj, trace=False)
    return out



# revision 56
# speedup vs baseline: 1.1135x; 1.0318x over previous
"""Causal self-attention (B=4, T=2048, C=1024, H=16, Dh=64) on 8 trn2 NeuronCores.

Sharding: core i <-> (batch b = i//2, head-group g = i%2). Each core computes
8 heads of one batch end-to-end (qkv slice, causal attention, partial output
projection); the host sums the head-group/pair-couple partials per batch and
adds bproj. No device collectives.

x arrives host-pretransposed as xT[C, T] (fp16), so qkv matmuls stream it
directly with the contraction dim on partitions -- no on-device transposes.
Attention uses the transposed-scores layout sT[tk, tq]: softmax denominators
come out of the PV matmul via an extra ones column interleaved into Wv, and
are broadcast across partitions with a partition-step-0 SBUF->SBUF DMA.
Partial projection outputs are written fp16 and summed on the host.
"""

import numpy as np

import concourse.bass as bass
import concourse.tile as tile
from concourse import bacc, mybir
from concourse.bass_utils import run_bass_kernel_spmd

F32 = mybir.dt.float32
F32R = mybir.dt.float32r
F16 = mybir.dt.float16

N_CORES = 8
B, T, C = 4, 2048, 1024
NH_TOT, D = 16, 64
F = 512            # features per core (8 heads)
NH = 8             # local heads
NPAIR = 4          # head pairs (128 feats each)
CCH = C // 128     # 8 contraction chunks
NTT = T // 128     # 16 t tiles
NTB = T // 512     # 4 t blocks (qkv production)
NQB = T // 512     # 4 q blocks (attention)
VW = NH * (D + 1)  # 520: augmented v width
ADD = mybir.AluOpType.add
MULT = mybir.AluOpType.mult


def _emit(tc, aps):
    from contextlib import ExitStack
    nc = tc.nc
    x, wq, wk, wva, bq, bk, wp = (
        aps["x"], aps["wq"], aps["wk"], aps["wva"], aps["bq"], aps["bk"],
        aps["wp"])
    out_ab = [aps["out_pa"], aps["out_pb"]]

    # ---- pools (all coexist; ~210KB/partition total) ----
    ctx = ExitStack()
    pp_qk = ctx.enter_context(tc.tile_pool(name="ps_qk", bufs=2, space="PSUM"))
    pp_s = ctx.enter_context(tc.tile_pool(name="ps_s", bufs=2, space="PSUM"))
    pp_pv = ctx.enter_context(tc.tile_pool(name="ps_pv", bufs=1, space="PSUM"))
    po_v = ctx.enter_context(tc.tile_pool(name="v_all", bufs=1))
    po_mask = ctx.enter_context(tc.tile_pool(name="mask", bufs=1))
    po_wv = ctx.enter_context(tc.tile_pool(name="wv", bufs=16))
    po_qkt = ctx.enter_context(tc.tile_pool(name="qkT", bufs=2))
    po_bias = ctx.enter_context(tc.tile_pool(name="bias", bufs=1))
    po_misc = ctx.enter_context(tc.tile_pool(name="misc", bufs=3))
    po_xt = ctx.enter_context(tc.tile_pool(name="xT", bufs=1))
    po_wqk = ctx.enter_context(tc.tile_pool(name="wqk", bufs=2))
    po_yt = ctx.enter_context(tc.tile_pool(name="yT", bufs=4))
    po_exp = ctx.enter_context(tc.tile_pool(name="expT", bufs=4))
    po_rec = ctx.enter_context(tc.tile_pool(name="recip", bufs=3))
    po_den = ctx.enter_context(tc.tile_pool(name="den", bufs=2))
    po_ytmp = ctx.enter_context(tc.tile_pool(name="ytmp", bufs=2))
    po_wp = ctx.enter_context(tc.tile_pool(name="wp", bufs=4))

    # emask[p, w] = -6e4 if w < 384 + p else 0: sliced at [384-off, 512)
    # it masks the diagonal 128-col triangle plus the garbage strip, and
    # is ACCUMULATED onto scores by the PE itself (identity stationary)
    # so the softmax chain never hops through the vector engine
    emask_sb = po_mask.tile([128, 512], F16, tag="emask")
    nc.scalar.dma_start(out=emask_sb[:], in_=aps["emask"][:])
    ident = po_bias.tile([128, 128], F16, tag="ident")
    nc.scalar.dma_start(out=ident[:], in_=aps["identin"][:])
    ones_sb = po_bias.tile([128, 64], F16, tag="ones64")
    nc.scalar.dma_start(out=ones_sb[:], in_=aps["ones64"][:])
    # bva broadcast to all 128 partitions straight from DRAM
    bva_bc = po_bias.tile([128, VW], F32, tag="bva_bc")
    bva2 = aps["bva2"]
    nc.scalar.dma_start(out=bva_bc[:], in_=bass.AP(
        tensor=bva2.tensor, offset=bva2.offset,
        ap=[[0, 128]] + [list(a) for a in bva2.ap[1:]]))

    # ---- phase 0: pair-0 qkv weights go first on the sync ring; the
    # 4MB xT is split across BOTH HWDGE rings (even chunks sync, odd
    # chunks scalar) for 2x delivery rate, wv after
    xT = [po_xt.tile([128, T], F16, tag=f"xT{c}", name=f"xT{c}")
          for c in range(CCH)]
    wv_sb = [[None] * CCH, [None] * CCH]

    def load_xt():
        # t-block-major so pair-0 qkv (t block 0) unblocks after ~1MB
        # instead of the full 4MB; wv quarters ride between blocks so
        # the first v units unblock progressively too
        for tb in range(NTB):
            tsl = slice(tb * 512, (tb + 1) * 512)
            for c in range(CCH):
                eng = nc.sync if c % 2 == 0 else nc.scalar
                eng.dma_start(out=xT[c][:, tsl],
                              in_=x[c * 128:(c + 1) * 128, tsl])
            half, part = divmod(tb, 2)
            cs = slice(half * 260, half * 260 + 260)
            for c in range(part * 4, part * 4 + 4):
                wt = po_wv.tile([128, 260], F16, tag="wv")
                nc.scalar.dma_start(out=wt[:],
                                    in_=wva[c * 128:(c + 1) * 128, cs])
                wv_sb[half][c] = wt

    def load_wv():
        pass

    # ---- phase 0b: v (augmented with ones columns, all 8 heads) ----
    # half 0 = heads 0-3 (pairs 0,1), half 1 = heads 4-7 (pairs 2,3);
    # half 1 production overlaps pair-0 attention. tt pairs alternate
    # psum banks so consecutive matmuls never accumulate into the same
    # bank back-to-back
    v_all = [po_v.tile([128, VW], F16, tag=f"v{tt}", name=f"v{tt}")
             for tt in range(NTT)]

    def v_units(half, split=1):
        cs = slice(half * 260, half * 260 + 260)
        units = []
        for tt0 in range(0, NTT, 2):
            stt = {}

            def part(tt0=tt0, cs=cs, half=half, stt=stt, c0=0, c1=CCH,
                     fin=True):
                if c0 == 0:
                    stt["ps0"] = pp_qk.tile([128, 260], F32, tag="qk",
                                            name="ps0")
                    stt["ps1"] = pp_qk.tile([128, 260], F32, tag="qk",
                                            name="ps1")
                ps0, ps1 = stt["ps0"], stt["ps1"]
                for c in range(c0, c1):
                    nc.tensor.matmul(
                        ps0[:], xT[c][:, tt0 * 128:(tt0 + 1) * 128],
                        wv_sb[half][c][:], start=(c == 0),
                        stop=(c == CCH - 1))
                    nc.tensor.matmul(
                        ps1[:], xT[c][:, (tt0 + 1) * 128:(tt0 + 2) * 128],
                        wv_sb[half][c][:], start=(c == 0),
                        stop=(c == CCH - 1))
                if fin:
                    nc.vector.tensor_add(v_all[tt0][:, cs], ps0[:],
                                         bva_bc[:, cs])
                    nc.vector.tensor_add(v_all[tt0 + 1][:, cs], ps1[:],
                                         bva_bc[:, cs])

            if split == 1:
                units.append(part)
            else:
                from functools import partial
                units.append(partial(part, c0=0, c1=4, fin=False))
                units.append(partial(part, c0=4, c1=CCH, fin=True))
        return units

    # ---- per head pair: qkv -> attention -> partial proj ----
    # Emitted as interleaved work units so the PE instruction stream mixes
    # next-pair qkv (and couple proj) matmuls between attention groups --
    # engines are in-order, so a blocked exp-wait would otherwise stall
    # ready qkv work behind it.

    def prep_qkv(pair):
        psl = slice(pair * 128, (pair + 1) * 128)
        # one [128, 8*256] tile: chunk c at cols [256c, 256c+128) = wq,
        # [256c+128, 256(c+1)) = wk. Loaded with TWO strided DMAs (the
        # 16 little per-chunk DMAs serialize ~1us each on the ring)
        wqk_all = po_wqk.tile([128, 256 * CCH], F16, tag="wqk",
                              name="wqk_all")
        dap = wqk_all[:]
        dstep = dap.ap[1][0]
        for which, w in ((0, wq), (1, wk)):
            src = w[:, psl]
            sstep_r, sstep_e = src.ap[0][0], src.ap[1][0]
            nc.sync.dma_start(
                out=bass.AP(
                    tensor=dap.tensor,
                    offset=dap.offset + which * 128 * dstep,
                    ap=[list(dap.ap[0]),
                        [256 * dstep, CCH], [dstep, 128]]),
                in_=bass.AP(
                    tensor=src.tensor, offset=src.offset,
                    ap=[[sstep_r, 128], [sstep_r * 128, CCH],
                        [sstep_e, 128]]))
        wqk_c = [wqk_all[:, 256 * c:256 * (c + 1)] for c in range(CCH)]
        bq_sb = po_bias.tile([128, 1], F32, tag=f"bq{pair}", name=f"bq{pair}")
        nc.sync.dma_start(out=bq_sb[:], in_=bq[psl, :])
        bk_sb = po_bias.tile([128, 1], F32, tag=f"bk{pair}", name=f"bk{pair}")
        nc.sync.dma_start(out=bk_sb[:], in_=bk[psl, :])
        qT = po_qkt.tile([128, T], F16, tag="qT", name="qT")
        kT = po_qkt.tile([128, T], F16, tag="kT", name="kT")
        return dict(wqk=wqk_c, bq=bq_sb, bk=bk_sb, qT=qT, kT=kT)

    def qkv_units(st8, split=1):
        # split=2 yields two micro-closures per t-block (for weaving
        # between attention steps); psum tile lifetime spans the pair,
        # so micros of one t-block must stay adjacent in their stream
        units = []
        for tb in range(NTB):
            tsl = slice(tb * 512, (tb + 1) * 512)
            stt = {}

            def half(tb=tb, tsl=tsl, stt=stt, c0=0, c1=CCH, fin=True):
                if c0 == 0:
                    stt["psq"] = pp_qk.tile([128, 512], F32, tag="qk",
                                            name="psq")
                    stt["psk"] = pp_qk.tile([128, 512], F32, tag="qk",
                                            name="psk")
                psq, psk = stt["psq"], stt["psk"]
                # q/k matmuls interleaved so consecutive matmuls target
                # alternating psum banks
                for c in range(c0, c1):
                    nc.tensor.matmul(psq[:], st8["wqk"][c][:, 0:128],
                                     xT[c][:, tsl],
                                     start=(c == 0), stop=(c == CCH - 1))
                    nc.tensor.matmul(psk[:], st8["wqk"][c][:, 128:256],
                                     xT[c][:, tsl],
                                     start=(c == 0), stop=(c == CCH - 1))
                if fin:
                    # psum*1/sqrt(D) + bq/sqrt(D)  (bq pre-scaled on host)
                    nc.vector.tensor_scalar(
                        out=st8["qT"][:, tsl], in0=psq[:], scalar1=0.125,
                        scalar2=st8["bq"][:], op0=MULT, op1=ADD)
                    nc.vector.tensor_scalar(
                        out=st8["kT"][:, tsl], in0=psk[:],
                        scalar1=st8["bk"][:], scalar2=None, op0=ADD)

            if split == 1:
                units.append(half)
            else:
                from functools import partial
                units.append(partial(half, c0=0, c1=4, fin=False))
                units.append(partial(half, c0=4, c1=CCH, fin=True))
        return units

    def attn_micros(st8, yt):
        # One q-block unit computes BOTH heads of the pair: the two
        # score matmuls contract over disjoint 64-partition halves
        # (rows 0-63 = head hl0, rows 64-127 = head hl1) so the PE runs
        # them CONCURRENTLY in separate row groups. One st tile packs
        # [hl0 scores | hl1 scores] so a single wide exp covers both.
        # Returns kt-granular micro-closures (for weaving fillers into
        # the exp-latency slack) plus per-unit end indices for gating.
        qT, kT = st8["qT"], st8["kT"]
        micros = []
        ends = []
        for qb in range(NQB):
            nkt = 4 * qb + 4
            ustate = {}

            def emit_scores(kt, st, qb=qb):
                j = kt - 4 * qb
                off = 128 * j if j > 0 else 0
                diag = j >= 0
                ktw = slice(kt * 128, (kt + 1) * 128)
                qw = slice(qb * 512 + off, (qb + 1) * 512)
                nc.tensor.matmul(st[:, off:512], kT[0:64, ktw],
                                 qT[0:64, qw], start=True, stop=not diag)
                if diag:
                    # accumulate the causal mask (and, for hl1 below, the
                    # garbage strip) on the PE: identity stationary x
                    # constant emask moving
                    nc.tensor.matmul(st[:, off:off + 128], ident[:],
                                     emask_sb[:, 384:512],
                                     start=False, stop=True)
                nc.tensor.matmul(st[:, 512 + off:1024], kT[64:128, ktw],
                                 qT[64:128, qw], start=True, stop=not diag)
                if diag:
                    # same 128-col triangle as hl0: the garbage strip
                    # [512, 512+off) is skipped by the 3D-AP exp below
                    nc.tensor.matmul(
                        st[:, 512 + off:512 + off + 128], ident[:],
                        emask_sb[:, 384:512], start=False, stop=True)
                return off

            def emit_exp_pv(kt, st, off, qb=qb, nkt=nkt, ustate=ustate):
                pair = st8["pair"]
                vslA = slice((pair * 2) * 65, (pair * 2) * 65 + 65)
                vslB = slice((pair * 2 + 1) * 65, (pair * 2 + 1) * 65 + 65)
                et = po_exp.tile([128, 1024], F16, tag="expT", name="et")
                if off > 0:
                    # two equal-width windows [off,512) and [512+off,1024)
                    # via a 3D AP -- the garbage strip between the head
                    # windows is never touched
                    w = 512 - off
                    eap, sap = et[:], st[:]
                    estep, sstep = eap.ap[1][0], sap.ap[1][0]
                    nc.scalar.activation(
                        bass.AP(tensor=et.tensor,
                                offset=eap.offset + off * estep,
                                ap=[list(eap.ap[0]), [512 * estep, 2],
                                    [estep, w]]),
                        bass.AP(tensor=st.tensor,
                                offset=sap.offset + off * sstep,
                                ap=[list(sap.ap[0]), [512 * sstep, 2],
                                    [sstep, w]]),
                        mybir.ActivationFunctionType.Exp)
                else:
                    nc.scalar.activation(
                        et[:, 0:1024], st[:, 0:1024],
                        mybir.ActivationFunctionType.Exp)
                nc.tensor.matmul(ustate["pvA"][0:65, off:512],
                                 v_all[kt][:, vslA], et[:, off:512],
                                 start=(kt == 0), stop=(kt == nkt - 1))
                nc.tensor.matmul(ustate["pvB"][0:65, off:512],
                                 v_all[kt][:, vslB], et[:, 512 + off:1024],
                                 start=(kt == 0), stop=(kt == nkt - 1))

            def step(kt, qb=qb, nkt=nkt, ustate=ustate, es=emit_scores,
                     ep=emit_exp_pv):
                if kt == 0:
                    ustate["pvA"] = pp_pv.tile([128, 512], F32, tag="pvA",
                                               name="pvA")
                    ustate["pvB"] = pp_pv.tile([128, 512], F32, tag="pvB",
                                               name="pvB")
                if kt < nkt:
                    st = pp_s.tile([128, 1024], F32, tag="s", name="st")
                    off = es(kt, st, qb=qb)
                    pend = ustate.get("pend")
                    if pend is not None:
                        ep(*pend)
                    ustate["pend"] = (kt, st, off)
                else:
                    ep(*ustate["pend"])
                    ustate["pend"] = None

            def tail(hl, qb=qb, ustate=ustate):
                # den row 64 -> SBUF -> PE ones-broadcast into a borrowed
                # psum bank (partitions 0-63) -> recip -> mul. No DMA in
                # the chain that gates pv-bank reuse of the next q-block.
                qsl = slice(qb * 512, (qb + 1) * 512)
                pv = ustate["pvA"] if hl == 0 else ustate["pvB"]
                den = po_den.tile([128, 512], F16, tag="den", name="den")
                nc.vector.tensor_copy(den[64:65, :], pv[64:65, :])
                recps = pp_qk.tile([128, 512], F32, tag="qk", name="recps")
                nc.tensor.matmul(recps[0:64, :], ones_sb[64:65, :],
                                 den[64:65, :], start=True, stop=True,
                                 tile_position=(64, 0))
                rec = po_rec.tile([128, 512], F32, tag="recip", name="rec")
                nc.vector.reciprocal_approx_fast(rec[0:64, :],
                                                 recps[0:64, :])
                if hl == 0:
                    nc.vector.tensor_mul(yt[0:64, qsl], pv[0:64, :],
                                         rec[0:64, :])
                else:
                    # engines can't cross partitions; bounce via DMA
                    ytmp = po_ytmp.tile([128, 512], F16, tag="ytmp",
                                        name="ytmp")
                    nc.vector.tensor_mul(ytmp[0:64, :], pv[0:64, :],
                                         rec[0:64, :])
                    nc.gpsimd.dma_start(out=yt[64:128, qsl],
                                        in_=ytmp[0:64, :])

            from functools import partial
            for kt in range(nkt):
                micros.append(partial(step, kt))
            def last(ustate=ustate, step=step, tail=tail, nkt=nkt):
                step(nkt)
                tail(0)
            micros.append(last)
            micros.append(partial(tail, 1))
            ends.append(len(micros) - 1)
        return micros, ends

    def prep_proj(couple):
        wp_sb = []
        for pq in range(2):
            for cb in range(2):
                prow = (couple * 2 + pq) * 128
                wt = po_wp.tile([128, 512], F16, tag="wp", name="wpt")
                nc.sync.dma_start(
                    out=wt[:],
                    in_=wp[prow:prow + 128, cb * 512:(cb + 1) * 512])
                wp_sb.append(wt)
        return wp_sb

    def proj_micros(couple, wp_sb, yts, tts):
        out_p = out_ab[couple]
        micros = []
        for tt in tts:
            def micro(tt=tt):
                ot = po_misc.tile([128, C], F16, tag="misc", name="ot")
                ps0 = pp_qk.tile([128, 512], F32, tag="qk", name="pp0")
                ps1 = pp_qk.tile([128, 512], F32, tag="qk", name="pp1")
                pss = (ps0, ps1)
                # pq outer: the y stationary is reused across the two
                # cb matmuls, which alternate psum banks
                for pq in range(2):
                    for cb in range(2):
                        nc.tensor.matmul(
                            pss[cb][:],
                            yts[pq][:, tt * 128:(tt + 1) * 128],
                            wp_sb[pq * 2 + cb][:],
                            start=(pq == 0), stop=(pq == 1))
                for cb in range(2):
                    nc.vector.tensor_copy(
                        ot[:, cb * 512:(cb + 1) * 512], pss[cb][:])
                nc.sync.dma_start(
                    out=out_p[tt * 128:(tt + 1) * 128, :], in_=ot[:])
            micros.append(micro)
        return micros

    def round_robin(*streams):
        streams = [list(s) for s in streams if s]
        while any(streams):
            for s in streams:
                if s:
                    s.pop(0)()

    def weave(primary, fillers, gates=None, boost=()):
        # Spread filler micro-closures evenly between primary ones;
        # gates[i] = index into primary that must already be emitted
        # before fillers[i] may run. boost = primary indices (attention
        # unit tails) after which extra fillers are popped to cover the
        # den-bounce chain that gates pv-bank reuse.
        nf, npr = len(fillers), len(primary)
        r = nf / npr if npr else 0.0
        acc, fi = 0.0, 0
        boost = set(boost)
        for pi, u in enumerate(primary):
            u()
            acc += r
            if pi in boost:
                acc += 2.0
            while (fi < nf and acc >= 1.0
                   and (gates is None or gates[fi] <= pi)):
                fillers[fi]()
                fi += 1
                acc -= 1.0
        while fi < nf:
            fillers[fi]()
            fi += 1

    pair_state = []
    yts = []
    st0 = prep_qkv(0)
    st0["pair"] = 0
    pair_state.append(st0)
    load_xt()
    load_wv()
    # pair-0 qkv races ahead of v-half0 so pair-0 attention (which only
    # needs half-0 v) starts as early as possible; v-half1 (for pairs
    # 2,3) overlaps pair-0 attention
    round_robin(qkv_units(st0), v_units(0))
    # p0: attn0 + (qkv1, v-half1) fillers
    yt0 = po_yt.tile([128, T], F16, tag="yT", name="yt0")
    yts.append(yt0)
    am0, ends0 = attn_micros(pair_state[0], yt0)
    st1 = prep_qkv(1)
    st1["pair"] = 1
    pair_state.append(st1)
    f0 = qkv_units(st1) + v_units(1)
    weave(am0, f0, [-1] * len(f0), boost=ends0)

    # p1: attn1 + (qkv2, first half of couple-0 proj) fillers
    yt1 = po_yt.tile([128, T], F16, tag="yT", name="yt1")
    yts.append(yt1)
    am1, ends1 = attn_micros(pair_state[1], yt1)
    st2 = prep_qkv(2)
    st2["pair"] = 2
    pair_state.append(st2)
    wp_sb0 = prep_proj(0)
    f1 = qkv_units(st2)
    g1 = [-1] * len(f1)
    f1 += proj_micros(0, wp_sb0, yts[0:2], range(0, 8))
    g1 += [ends1[tt // 4] for tt in range(0, 8)]
    weave(am1, f1, g1, boost=ends1)

    # p2+p3 merged: one weave so fillers flow across the boundary
    yt2 = po_yt.tile([128, T], F16, tag="yT", name="yt2")
    yt3 = po_yt.tile([128, T], F16, tag="yT", name="yt3")
    yts += [yt2, yt3]
    st3 = prep_qkv(3)
    st3["pair"] = 3
    pair_state.append(st3)
    am2, ends2 = attn_micros(pair_state[2], yt2)
    am3, ends3 = attn_micros(pair_state[3], yt3)
    wp_sb1 = prep_proj(1)
    am23 = am2 + am3
    ends23 = list(ends2) + [len(am2) + e for e in ends3]
    f2 = qkv_units(st3)
    g2 = [-1] * len(f2)
    f2 += proj_micros(0, wp_sb0, yts[0:2], range(8, 16))
    g2 += [-1] * 8
    f2 += proj_micros(1, wp_sb1, yts[2:4], range(0, 16))
    g2 += [len(am2) + ends3[tt // 4] for tt in range(0, 16)]
    weave(am23, f2, g2, boost=ends23)

    ctx.close()


_CACHE = {}


def _build():
    if "nc" in _CACHE:
        return _CACHE["nc"]
    nc = bacc.Bacc("TRN2", target_bir_lowering=False, debug=False,
                   enable_asserts=True, num_devices=N_CORES)
    aps = {
        "x": nc.dram_tensor("x", [C, T], F16, kind="ExternalInput").ap(),
        "wq": nc.dram_tensor("wq", [C, F], F16, kind="ExternalInput").ap(),
        "wk": nc.dram_tensor("wk", [C, F], F16, kind="ExternalInput").ap(),
        "wva": nc.dram_tensor("wva", [C, VW], F16, kind="ExternalInput").ap(),
        "bq": nc.dram_tensor("bq", [F, 1], F32, kind="ExternalInput").ap(),
        "bk": nc.dram_tensor("bk", [F, 1], F32, kind="ExternalInput").ap(),
        "bva2": nc.dram_tensor("bva2", [1, VW], F32, kind="ExternalInput").ap(),
        "wp": nc.dram_tensor("wp", [F, C], F16, kind="ExternalInput").ap(),
        "emask": nc.dram_tensor("emask", [128, 512], F16,
                                kind="ExternalInput").ap(),
        "identin": nc.dram_tensor("identin", [128, 128], F16,
                                  kind="ExternalInput").ap(),
        "ones64": nc.dram_tensor("ones64", [128, 64], F16,
                                 kind="ExternalInput").ap(),
        "out_pa": nc.dram_tensor("out_pa", [T, C], F16,
                                 kind="ExternalOutput").ap(),
        "out_pb": nc.dram_tensor("out_pb", [T, C], F16,
                                 kind="ExternalOutput").ap(),
    }
    with tile.TileContext(nc) as tc:
        _emit(tc, aps)
    nc.compile()
    _CACHE["nc"] = nc
    return nc


def _make_in_maps(x, Wqkv, bqkv, Wproj):
    x = np.asarray(x, dtype=np.float32)
    Wqkv = np.asarray(Wqkv, dtype=np.float32)
    bqkv = np.asarray(bqkv, dtype=np.float32)
    Wproj = np.asarray(Wproj, dtype=np.float32)

    # emask[p, w] = -6e4 if w < 384 + p else 0 (accumulated onto scores
    # by the PE; -6e4 stays within f16 range and exp() underflows to 0)
    p_idx = np.arange(128)[:, None]
    w_idx = np.arange(512)[None, :]
    emask = np.where(w_idx < 384 + p_idx, -6e4, 0.0).astype(np.float16)

    in_maps = []
    for core in range(N_CORES):
        b, g = divmod(core, 2)
        q0, k0, v0 = 512 * g, C + 512 * g, 2 * C + 512 * g
        wva = np.zeros((C, VW), dtype=np.float32)
        bva = np.zeros((1, VW), dtype=np.float32)
        for h in range(NH):
            src = v0 + D * h
            dst = 65 * h
            # per-head layout [v(64), one]
            wva[:, dst:dst + 64] = Wqkv[:, src:src + 64]
            bva[0, dst:dst + 64] = bqkv[src:src + 64]
            bva[0, dst + 64] = 1.0
        in_maps.append({
            "x": np.ascontiguousarray(x[b].T).astype(np.float16),
            "wq": np.ascontiguousarray(Wqkv[:, q0:q0 + F]).astype(np.float16),
            "wk": np.ascontiguousarray(Wqkv[:, k0:k0 + F]).astype(np.float16),
            "wva": wva.astype(np.float16),
            "bq": np.ascontiguousarray(bqkv[q0:q0 + F].reshape(F, 1) * 0.125),
            "bk": np.ascontiguousarray(bqkv[k0:k0 + F].reshape(F, 1)),
            "bva2": bva,
            "wp": np.ascontiguousarray(Wproj[512 * g:512 * g + F, :]).astype(np.float16),
            "emask": emask,
            "identin": np.eye(128, dtype=np.float16),
            "ones64": np.ones((128, 64), dtype=np.float16),
        })
    return in_maps


def run_sharded(x, Wqkv, bqkv, Wproj, bproj, trace=False):
    nc = _build()
    in_maps = _make_in_maps(x, Wqkv, bqkv, Wproj)
    res = run_bass_kernel_spmd(nc, in_maps, core_ids=list(range(N_CORES)),
                               trace=trace)
    bproj = np.asarray(bproj, dtype=np.float32)
    out = np.empty((B, T, C), dtype=np.float32)
    for b in range(B):
        acc = bproj[None, :].astype(np.float32).repeat(T, axis=0)
        for core in (2 * b, 2 * b + 1):
            acc = (acc + res.results[core]["out_pa"].astype(np.float32)
                   + res.results[core]["out_pb"].astype(np.float32))
        out[b] = acc
    return out, res


def kernel(x, Wqkv, bqkv, Wproj, bproj):
    out, _ = run_sharded(x, Wqkv, bqkv, Wproj, bproj, trace=False)
    return out

